# revision 33
# baseline (speedup 1.0000x reference)
"""Canny edge detector on 8 Trainium2 NeuronCores — pure data-parallel (1 image/core).

Pipeline per core (image 1024x1024 f32):
  1. 5x5 Gaussian blur (separable: vertical then horizontal 5-tap, exact f32)
  2. Sobel gx, gy (separable 3-taps)
  3. NMS using squared magnitudes (no sqrt / atan2 needed)
  4. Hysteresis: 16 iterations of 3x3 binary dilation masked by weak, on
     bit-packed state (32 px/word) with per-row gutter words.

Layout: "multirow" — partition p holds image rows [8p+d] in its free
dimension, row pitch 1028 (2 zero gutter cols each side) so ALL 8-neighbor
shifts are free-dim AP offsets.  Vertical halos come from overlapping HBM
loads (img) and SBUF->SBUF DMA halo refreshes (blurred, msq, packed state).

Engine facts (BIR-verifier-probed): Pool/GPSIMD supports ONLY f32
tensor_tensor add/sub/mult, tensor_single_scalar mult/add/max/compares,
copy, memset, iota.  No STT, no TT max/compare, no bitwise/shifts.
DVE does everything; Act does single-input activations (copy-scale,
square, relu).  So:
  - TT add/sub/mult ops column-split DVE|Pool at 672 (rates 1.042 vs 1.984)
  - single-scalar ops split at 745 (0.521 vs 1.389)
  - fused STT combines (a*s + b) run on DVE for cols [0:782], decomposed
    TSS-mult + TT-add on Pool for the rest
  - NMS pair maxes: DVE TT-max [0:810], Pool sub + Act relu + Pool add
    beyond (max(a,b) = b + relu(a-b); <=1-ulp rounding, flips only exact
    NMS ties — probability ~1e-12 per pixel)
  - hysteresis bitwise loop is DVE-only (hardware restriction), with
    interior-first iterations after each halo exchange to hide DMA latency
"""
import numpy as np

import concourse.bass as bass
import concourse.mybir as mybir
from concourse.tile import TileContext
from concourse.bass_utils import run_bass_kernel_spmd

P = 128          # partitions
R = 8            # image rows per partition
H = W = 1024
RP = 1028        # row pitch (2 gutter cols + 1024 data + 2 gutter cols)
DOF = 2          # data column offset within a row slot

# packed layout: 32 px/word -> 32 data words + 1 zero gutter word per row
PW = 33
NDW = 32

# hysteresis packed tile: 1 margin + (J halo + 8 own + J halo) data rows + 1 margin
HJ = 2           # halo rows == refresh cadence (iterations between halo refreshes)
HNR = 2 + 8 + 2 * HJ
HD0 = 1          # first data row (halo-top) in packed tiles
HOWN = 1 + HJ    # first own row in packed tiles

F32 = mybir.dt.float32
U32 = mybir.dt.uint32
U16 = mybir.dt.uint16
I16 = mybir.dt.int16
I32 = mybir.dt.int32
I8 = mybir.dt.int8

# DVE | Pool column splits (DVE gets [0:r), Pool [r:W))
RA = 672     # TT add/sub/mult      (DVE 1.042 vs Pool 1.984 ns/elem)
RS = 745     # single-scalar ops    (DVE 0.521 vs Pool 1.389)
RC = 782     # STT combine vs Pool TSS-mult + TT-add  (1.042 vs 3.373)
RX = 810     # TT max vs Pool sub + Act relu + Pool add (1.042 vs 3.968)
RN = 840     # STT cmp vs Pool TSS-mult + TT-sub + TSS-cmp (1.042 vs 4.762)
RT = 390     # pack-tree level-1 split of 512 pair sums


def _f32_consts():
    ax = np.arange(5, dtype=np.float32) - np.float32(2.0)
    g = np.exp(-(ax ** 2) / np.float32(2.0)).astype(np.float32)
    g = (g / g.sum()).astype(np.float32)
    c1 = np.float32(np.tan(np.deg2rad(22.5)) ** 2)
    c2 = np.float32(np.tan(np.deg2rad(67.5)) ** 2)

    def sqrt_thresh(t):
        t = np.float32(t)
        x = np.float32(t) * np.float32(t)
        while np.sqrt(np.float32(x)) >= t:
            x = np.nextafter(x, np.float32(0.0), dtype=np.float32)
        while np.sqrt(np.float32(x)) < t:
            x = np.nextafter(x, np.float32(np.inf), dtype=np.float32)
        return np.float32(x)

    return g, c1, c2, sqrt_thresh(0.1), sqrt_thresh(0.2)


def build_canny(nc, tc, pool, img_d, out_d, stage=99):
    import os
    stage = int(os.environ.get("CANNY_STAGE", stage))
    from concourse.alu_op_type import AluOpType as A
    g, c1, c2, tlow, thigh = _f32_consts()
    ve = nc.vector
    gp = nc.gpsimd
    se = nc.scalar

    def bail():
        z = pool.tile([P, 8, W], F32, name="zz", tag="C")
        ve.memset(z[:, :, :], 0.0)
        nc.sync.dma_start(out=out_d.rearrange("(p r w) -> p r w", p=P, r=R),
                          in_=z[:, :, :])

    def sp(r):
        return ((ve, 0, r), (gp, r, W))

    def zero_gutters(eng, t, nr):
        eng.memset(t[:, 0:nr, 0:DOF], 0.0)
        eng.memset(t[:, 0:nr, DOF + W:RP], 0.0)

    def comb(mk_dst, mk_src, s):
        """dst += s*src: DVE fused STT on [0:RC]; Pool scales src in place
        (src must be dead afterwards) then adds, on [RC:W]."""
        ve.scalar_tensor_tensor(mk_dst(0, RC), mk_src(0, RC), float(s),
                                mk_dst(0, RC), op0=A.mult, op1=A.add)
        gp.tensor_single_scalar(mk_src(RC, W), mk_src(RC, W), float(s),
                                op=A.mult)
        gp.tensor_tensor(mk_dst(RC, W), mk_src(RC, W), mk_dst(RC, W),
                         op=A.add)

    # per-partition integer scalar constants for bitwise scalar_tensor_tensor
    cst = pool.tile([P, 4], U32, name="cst", tag="tcst")
    ve.memset(cst[:, 0:1], 1)
    ve.memset(cst[:, 1:2], 16)
    ve.memset(cst[:, 2:3], 31)
    C1A, C16A, C31A = cst[:, 0:1], cst[:, 1:2], cst[:, 2:3]

    # ---------------- load image (rows 8p-2 .. 8p+10) ----------------
    img = pool.tile([P, 12, W], F32, name="img", tag="A")
    # out-of-image halo rows must be zero; the loads below overwrite all but
    # partition 0 / 127 edges (compute ops cannot start at partition 127, so
    # full-partition memsets, one per engine, before the loads)
    ve.memset(img[:, 0:2, :], 0.0)
    gp.memset(img[:, 10:12, :], 0.0)  # full partitions: p127 can't be sliced

    img_rows = img_d.rearrange("(n w) -> n w", w=W)
    img_win = bass.AP(img_d, (R - 2) * W, [[R * W, P - 2], [W, 12], [1, W]])
    nc.sync.dma_start(out=img[1:P - 1, :, :], in_=img_win)
    nc.sync.dma_start(out=img[0:1, 2:12, :],
                      in_=img_rows[0:10, :].rearrange("(p r) w -> p r w", p=1))
    nc.sync.dma_start(out=img[P - 1:P, 0:10, :],
                      in_=img_rows[H - 10:H, :].rearrange("(p r) w -> p r w", p=1))

    # ---------------- vertical 5-tap blur -> blurv (own 8 rows) ----------------
    blurv = pool.tile([P, 8, RP], F32, name="blurv", tag="B")
    zero_gutters(ve, blurv, 8)
    pa1 = pool.tile([P, 8, W], F32, name="pa1", tag="C")
    pa2 = pool.tile([P, 8, W], F32, name="pa2", tag="F")
    for eng, c0, c1_ in sp(RA):
        eng.tensor_tensor(pa1[:, :, c0:c1_], img[:, 1:9, c0:c1_],
                          img[:, 3:11, c0:c1_], op=A.add)
        eng.tensor_tensor(pa2[:, :, c0:c1_], img[:, 0:8, c0:c1_],
                          img[:, 4:12, c0:c1_], op=A.add)
    se.mul(blurv[:, :, DOF:DOF + W], img[:, 2:10, :], float(g[2]))
    comb(lambda a, b: blurv[:, :, DOF + a:DOF + b],
         lambda a, b: pa1[:, :, a:b], g[1])
    comb(lambda a, b: blurv[:, :, DOF + a:DOF + b],
         lambda a, b: pa2[:, :, a:b], g[0])

    if stage <= 1:
        bail()
        return

    # ---------------- horizontal 5-tap blur -> blurred [10 rows, own at 1..9] ---
    blurred = pool.tile([P, 10, RP], F32, name="blurred", tag="A")
    pb1 = pool.tile([P, 8, W], F32, name="pb1", tag="C")
    pb2 = pool.tile([P, 8, W], F32, name="pb2", tag="F")
    for eng, c0, c1_ in sp(RA):
        eng.tensor_tensor(pb1[:, :, c0:c1_],
                          blurv[:, :, DOF + c0 - 1:DOF + c1_ - 1],
                          blurv[:, :, DOF + c0 + 1:DOF + c1_ + 1], op=A.add)
        eng.tensor_tensor(pb2[:, :, c0:c1_],
                          blurv[:, :, DOF + c0 - 2:DOF + c1_ - 2],
                          blurv[:, :, DOF + c0 + 2:DOF + c1_ + 2], op=A.add)
    se.mul(blurred[:, 1:9, DOF:DOF + W], blurv[:, :, DOF:DOF + W], float(g[2]))
    comb(lambda a, b: blurred[:, 1:9, DOF + a:DOF + b],
         lambda a, b: pb1[:, :, a:b], g[1])
    comb(lambda a, b: blurred[:, 1:9, DOF + a:DOF + b],
         lambda a, b: pb2[:, :, a:b], g[0])
    # halo refresh: row 0 <- p-1 own row 7 (tile row 8); row 9 <- p+1 own row 0
    ve.memset(blurred[:, 0:1, :], 0.0)
    ve.memset(blurred[:, 9:10, :], 0.0)
    nc.sync.dma_start(out=blurred[1:P, 0:1, DOF:DOF + W],
                      in_=blurred[0:P - 1, 8:9, DOF:DOF + W])
    nc.scalar.dma_start(out=blurred[0:P - 1, 9:10, DOF:DOF + W],
                        in_=blurred[1:P, 1:2, DOF:DOF + W])

    if stage <= 2:
        bail()
        return

    # ---------------- sobel vertical parts (own 8 rows) ----------------
    # wx = bl[r-1] + 2 bl[r] + bl[r+1] ; vy = bl[r+1] - bl[r-1]
    wx = pool.tile([P, 8, RP], F32, name="wx", tag="C")
    vy = pool.tile([P, 8, RP], F32, name="vy", tag="F")
    zero_gutters(ve, wx, 8)
    zero_gutters(gp, vy, 8)
    for eng, c0, c1_ in sp(RA):
        eng.tensor_tensor(wx[:, :, DOF + c0:DOF + c1_],
                          blurred[:, 0:8, DOF + c0:DOF + c1_],
                          blurred[:, 2:10, DOF + c0:DOF + c1_], op=A.add)
        eng.tensor_tensor(vy[:, :, DOF + c0:DOF + c1_],
                          blurred[:, 2:10, DOF + c0:DOF + c1_],
                          blurred[:, 0:8, DOF + c0:DOF + c1_], op=A.subtract)
    # wx += 2*bl(center); Pool side scales blurred rows 1:9 in place (dead after)
    comb(lambda a, b: wx[:, :, DOF + a:DOF + b],
         lambda a, b: blurred[:, 1:9, DOF + a:DOF + b], 2.0)

    # ---------------- sobel horizontal parts ----------------
    gx = pool.tile([P, 8, RP], F32, name="gx", tag="B")
    gy = pool.tile([P, 8, RP], F32, name="gy", tag="A")
    gx_d = gx[:, :, DOF:DOF + W]
    gy_d = gy[:, :, DOF:DOF + W]
    for eng, c0, c1_ in sp(RA):
        eng.tensor_tensor(gx[:, :, DOF + c0:DOF + c1_],
                          wx[:, :, DOF + c0 + 1:DOF + c1_ + 1],
                          wx[:, :, DOF + c0 - 1:DOF + c1_ - 1], op=A.subtract)
        eng.tensor_tensor(gy[:, :, DOF + c0:DOF + c1_],
                          vy[:, :, DOF + c0 - 1:DOF + c1_ - 1],
                          vy[:, :, DOF + c0 + 1:DOF + c1_ + 1], op=A.add)
    # gy += 2*vy; Pool side scales vy in place (dead after)
    comb(lambda a, b: gy[:, :, DOF + a:DOF + b],
         lambda a, b: vy[:, :, DOF + a:DOF + b], 2.0)

    if stage <= 3:
        bail()
        return

    # ---------------- sign of gx*gy, squares, msq ----------------
    # diagonal-class mask: sm = (gx*gy < 0).  Product underflow to +-0 only
    # happens when msq is far below the weak threshold, where the NMS
    # direction choice can't affect the output.
    smw = pool.tile([P, 8, W], F32, name="smw", tag="C")
    sm = pool.tile([P, 8, W], I8, name="sm", tag="G2")
    for eng, c0, c1_ in sp(RA):
        eng.tensor_tensor(smw[:, :, c0:c1_], gx[:, :, DOF + c0:DOF + c1_],
                          gy[:, :, DOF + c0:DOF + c1_], op=A.mult)
    for eng, c0, c1_ in sp(RS):
        eng.tensor_single_scalar(sm[:, :, c0:c1_], smw[:, :, c0:c1_], 0.0,
                                 op=A.is_lt)

    se.square(gx_d, gx_d)   # sqx
    se.square(gy_d, gy_d)   # sqy
    sqx, sqy = gx, gy
    sqx_d, sqy_d = gx_d, gy_d

    # msq [10 rows, own at 1..9] with DMA halo refresh (before nb0/nb2 so the
    # Pool decompositions may clobber sqx afterwards)
    msq = pool.tile([P, 10, RP], F32, name="msq", tag="F")
    zero_gutters(ve, msq, 10)
    for eng, c0, c1_ in sp(RA):
        eng.tensor_tensor(msq[:, 1:9, DOF + c0:DOF + c1_],
                          sqx[:, :, DOF + c0:DOF + c1_],
                          sqy[:, :, DOF + c0:DOF + c1_], op=A.add)
    ve.memset(msq[:, 0:1, :], 0.0)
    ve.memset(msq[:, 9:10, :], 0.0)
    nc.sync.dma_start(out=msq[1:P, 0:1, :], in_=msq[0:P - 1, 8:9, :])
    nc.scalar.dma_start(out=msq[0:P - 1, 9:10, :], in_=msq[1:P, 1:2, :])

    # direction classes (int8 0/1): nb0 = sqy < c1*sqx ; nb2 = sqy >= c2*sqx
    # DVE: fused STT on [0:RN].  Pool on [RN:W]: t = c*sqx; d = t - sqy;
    # mask = sign test (exact: f32 subtract has exact sign).
    nb0 = pool.tile([P, 8, W], I8, name="nb0", tag="G")
    nb2 = pool.tile([P, 8, W], I8, name="nb2", tag="Hh")
    sc = pool.tile([P, 8, W - RN], F32, name="sc", tag="SC",
                   padded_shape=[P, 8, W - RX])
    ve.scalar_tensor_tensor(nb0[:, :, 0:RN], sqx_d[:, :, 0:RN], float(c1),
                            sqy_d[:, :, 0:RN], op0=A.mult, op1=A.is_gt)
    ve.scalar_tensor_tensor(nb2[:, :, 0:RN], sqx_d[:, :, 0:RN], float(c2),
                            sqy_d[:, :, 0:RN], op0=A.mult, op1=A.is_le)
    gp.tensor_single_scalar(sc[:, :, :], sqx_d[:, :, RN:W], float(c1),
                            op=A.mult)
    gp.tensor_tensor(sc[:, :, :], sc[:, :, :], sqy_d[:, :, RN:W],
                     op=A.subtract)
    gp.tensor_single_scalar(nb0[:, :, RN:W], sc[:, :, :], 0.0, op=A.is_gt)
    # nb2 Pool side: clobber sqx in place (dead after this)
    gp.tensor_single_scalar(sqx_d[:, :, RN:W], sqx_d[:, :, RN:W], float(c2),
                            op=A.mult)
    gp.tensor_tensor(sqx_d[:, :, RN:W], sqx_d[:, :, RN:W],
                     sqy_d[:, :, RN:W], op=A.subtract)
    gp.tensor_single_scalar(nb2[:, :, RN:W], sqx_d[:, :, RN:W], 0.0,
                            op=A.is_le)

    if stage <= 4:
        bail()
        return

    # ---------------- NMS: directional pair maxes + predicated select ----------
    def msq_sh(dr, dj, c0=0, c1_=W):
        return msq[:, 1 + dr:9 + dr, DOF + dj + c0:DOF + dj + c1_]

    # Pair maxes all on DVE (Pool cannot do TT max, and decomposed
    # sub/relu/add chains stall the pred cascade).  While DVE runs the maxes
    # and the predicated-copy cascade, Pool packs the threshold planes
    # tw/ts = (msq >= t) into 16-bit halfword sums via the pairwise doubling
    # tree (TSS-mult + TT-add, Pool-legal; Act takes the odd-scales).
    # Packing distributes over AND, so afterwards DVE only packs
    # kb = (msq >= M) and bitwise-ANDs the packed words:
    #   pack(kb AND t) = pack(kb) & pack(t)       (bits are independent)
    M = pool.tile([P, 8, W], F32, name="M", tag="B")
    m_d2 = pool.tile([P, 8, W], F32, name="m_d2", tag="A")
    m_ns = pool.tile([P, 8, W], F32, name="m_ns", tag="C")

    # packed halfword-sum planes (f32 sums 0..65535, exact)
    ptw = pool.tile([P, 8, 64], F32, name="ptw", tag="PTW")
    pts = pool.tile([P, 8, 64], F32, name="pts", tag="PTS")

    def pool_pack_tw():
        # quarter-column passes, entirely on Pool (TSS in-place odd-scales +
        # TT adds) so the tree never waits on another engine.
        mults = (2.0, 4.0, 16.0, 256.0)
        for q in range(4):
            th = pool.tile([P, 8, 256], F32, name="th", tag="SC")
            gp.tensor_single_scalar(th[:, :, :],
                                    msq_sh(0, 0, 256 * q, 256 * q + 256),
                                    float(tlow), op=A.is_ge)
            cur = th
            for li in range(4):
                n = 128 >> li
                cur_r = cur.rearrange("p r (j two) -> p r j two", two=2)
                gp.tensor_single_scalar(cur_r[:, :, :, 1], cur_r[:, :, :, 1],
                                        mults[li], op=A.mult)
                if li < 3:
                    nxt = pool.tile([P, 8, n], F32, name=f"pt{li}",
                                    tag=("SD", "SE", "SD")[li])
                    gp.tensor_tensor(nxt[:, :, :], cur_r[:, :, :, 1],
                                     cur_r[:, :, :, 0], op=A.add)
                    cur = nxt
                else:
                    gp.tensor_tensor(ptw[:, :, 16 * q:16 * q + 16],
                                     cur_r[:, :, :, 1], cur_r[:, :, :, 0],
                                     op=A.add)

    def pmax_dve(dst, a_dr, a_dj, b_dr, b_dj):
        ve.tensor_tensor(dst[:, :, :], msq_sh(a_dr, a_dj),
                         msq_sh(b_dr, b_dj), op=A.max)

    pool_pack_tw()
    pmax_dve(m_d2, -1, -1, 1, 1)
    pmax_dve(M, -1, 1, 1, -1)
    HALF = 512
    for c0, c1_ in ((0, HALF), (HALF, W)):
        ve.copy_predicated(M[:, :, c0:c1_], sm[:, :, c0:c1_],
                           m_d2[:, :, c0:c1_])
    m_ew = pool.tile([P, 8, W], F32, name="m_ew", tag="A")
    pmax_dve(m_ns, -1, 0, 1, 0)
    for c0, c1_ in ((0, HALF), (HALF, W)):
        ve.copy_predicated(M[:, :, c0:c1_], nb2[:, :, c0:c1_],
                           m_ns[:, :, c0:c1_])
    pmax_dve(m_ew, 0, 1, 0, -1)
    for c0, c1_ in ((0, HALF), (HALF, W)):
        ve.copy_predicated(M[:, :, c0:c1_], nb0[:, :, c0:c1_],
                           m_ew[:, :, c0:c1_])

    if stage <= 5:
        bail()
        return

    # ---------------- keep-plane pack + AND with threshold planes -------------
    ps = pool.tile([P, HNR, PW], U32, name="ps", tag="tps")
    pw_ = pool.tile([P, HNR, PW], U32, name="pw_", tag="tpw")
    gp.memset(ps[:, :, :], 0)
    gp.memset(pw_[:, :, :], 0)

    # kb = (msq >= M) via exact-sign subtract
    kb = pool.tile([P, 8, W], F32, name="kb", tag="A")
    for eng, c0, c1_ in sp(RA):
        eng.tensor_tensor(kb[:, :, c0:c1_], msq_sh(0, 0, c0, c1_),
                          M[:, :, c0:c1_], op=A.subtract)
    for eng, c0, c1_ in sp(RS):
        eng.tensor_single_scalar(kb[:, :, c0:c1_], kb[:, :, c0:c1_], 0.0,
                                 op=A.is_ge)

    # DVE trees (fused STT levels, per column half with small scratch):
    # one on kb, one on the strong-threshold plane ts
    def dve_tree(plane, out_plane):
        mults = (2.0, 4.0, 16.0, 256.0)
        for hh in (0, 1):
            cur = plane[:, :, 512 * hh:512 * hh + 512]
            n = 256
            tags = ("SC", "SD", "SC")
            for li in range(4):
                cur_r = cur.rearrange("p r (j two) -> p r j two", two=2)
                nxt = (pool.tile([P, 8, n], F32, name=f"kt{li}",
                                 tag=tags[li])
                       if li < 3 else out_plane[:, :, 32 * hh:32 * hh + 32])
                ve.scalar_tensor_tensor(nxt, cur_r[:, :, :, 1], mults[li],
                                        cur_r[:, :, :, 0],
                                        op0=A.mult, op1=A.add)
                cur = nxt
                n //= 2

    ts = pool.tile([P, 8, W], F32, name="ts", tag="B")
    for eng, c0, c1_ in sp(RS):
        eng.tensor_single_scalar(ts[:, :, c0:c1_], msq_sh(0, 0, c0, c1_),
                                 float(thigh), op=A.is_ge)
    pkb = pool.tile([P, 8, 64], F32, name="pkb", tag="PKB")
    dve_tree(kb, pkb)
    dve_tree(ts, pts)

    # convert the three halfword-sum planes to u32 and AND packed words
    ikb = pool.tile([P, 8, 64], U32, name="ikb", tag="SC")
    itw = pool.tile([P, 8, 64], U32, name="itw", tag="SD")
    its = pool.tile([P, 8, 64], U32, name="its", tag="SE")
    ve.tensor_copy(ikb[:, :, :], pkb[:, :, :])
    ve.tensor_copy(itw[:, :, :], ptw[:, :, :])
    ve.tensor_copy(its[:, :, :], pts[:, :, :])
    ve.tensor_tensor(itw[:, :, :], itw[:, :, :], ikb[:, :, :],
                     op=A.bitwise_and)
    ve.tensor_tensor(its[:, :, :], its[:, :, :], ikb[:, :, :],
                     op=A.bitwise_and)
    for t, srcw in ((pw_, itw), (ps, its)):
        hv = srcw.rearrange("p r (s two) -> p r s two", two=2)
        ve.scalar_tensor_tensor(t[:, HOWN:HOWN + 8, 0:NDW], hv[:, :, :, 1],
                                C16A, hv[:, :, :, 0],
                                op0=A.logical_shift_left, op1=A.bitwise_or)

    # ---------------- packed halos ----------------
    def refresh_halos(t):
        nc.sync.dma_start(out=t[1:P, HD0:HD0 + HJ, :],
                          in_=t[0:P - 1, HOWN + 8 - HJ:HOWN + 8, :])
        nc.scalar.dma_start(out=t[0:P - 1, HOWN + 8:HOWN + 8 + HJ, :],
                            in_=t[1:P, HOWN:HOWN + HJ, :])

    refresh_halos(pw_)
    refresh_halos(ps)

    if stage <= 6:
        bail()
        return

    # ---------------- 16 iterations of masked dilation (packed) --------------
    # Bitwise u32 ops are DVE-only on TRN2, so the whole packed loop runs on
    # the vector engine.  V has a zero gutter slot on each side so every
    # word-shift read stays in-tile.  Post-refresh iterations process
    # interior rows first so the halo-exchange DMA latency hides.
    nd = 8 + 2 * HJ
    Vd = pool.tile([P, HNR, NDW + 2], U32, name="Vd", tag="tV")  # gutter,0..31,gutter
    Hd = pool.tile([P, HNR, NDW], U32, name="Hd", tag="tH")
    ve.memset(Vd[:, :, :], 0)
    ve.memset(Hd[:, :, :], 0)

    def rsel(t, g, dr, w0, w1):
        # row-group selector: 'all' rows 1..12, 'core' 4..9, and the two
        # contiguous rim bands (STT ops only accept 2D/3D APs)
        if g == "all":
            return t[:, HD0 + dr:HD0 + nd + dr, w0:w1]
        if g == "core":
            return t[:, 4 + dr:10 + dr, w0:w1]
        if g == "rim1":
            return t[:, 1 + dr:4 + dr, w0:w1]
        return t[:, 10 + dr:13 + dr, w0:w1]

    def hyst_iter(groups=("all",)):
        for g in groups:
            V = rsel(Vd, g, 0, 1, NDW + 1)
            Hh = rsel(Hd, g, 0, 0, NDW)
            ve.tensor_tensor(V, rsel(ps, g, -1, 0, NDW),
                             rsel(ps, g, 1, 0, NDW), op=A.bitwise_or)
            ve.tensor_tensor(V, rsel(ps, g, 0, 0, NDW), V, op=A.bitwise_or)
            ve.scalar_tensor_tensor(Hh, V, C1A, V, op0=A.logical_shift_left,
                                    op1=A.bitwise_or)
            ve.scalar_tensor_tensor(Hh, V, C1A, Hh, op0=A.logical_shift_right,
                                    op1=A.bitwise_or)
            ve.scalar_tensor_tensor(Hh, rsel(Vd, g, 0, 0, NDW), C31A, Hh,
                                    op0=A.logical_shift_right, op1=A.bitwise_or)
            ve.scalar_tensor_tensor(Hh, rsel(Vd, g, 0, 2, NDW + 2), C31A, Hh,
                                    op0=A.logical_shift_left, op1=A.bitwise_or)
        for g in groups:
            ve.tensor_tensor(rsel(ps, g, 0, 0, NDW), rsel(Hd, g, 0, 0, NDW),
                             rsel(pw_, g, 0, 0, NDW), op=A.bitwise_and)

    n_iters = int(os.environ.get("CANNY_HYST_ITERS", 16))
    no_refresh = int(os.environ.get("CANNY_NO_REFRESH", 0))  # timing expt only
    hide = int(os.environ.get("CANNY_HIDE_REFRESH", 1))
    for it in range(n_iters):
        post_refresh = hide and it > 0 and it % HJ == 0 and not no_refresh
        hyst_iter(("core", "rim1", "rim2") if post_refresh else ("all",))
        if (it + 1) % HJ == 0 and it < n_iters - 1 and not no_refresh:
            refresh_halos(ps)

    if stage <= 7:
        bail()
        return

    # ---------------- unpack own rows -> f32 0/1 and store --------------------
    # ps own words viewed as u16 halfwords h (= pixels 16h..16h+15).  For
    # each bit position k: tub[k][h] = hw[h] << (15-k) (single-src imm-shift
    # TSS on packed u16 = 4x DVE mode), then sign test in the transposed
    # view.  Processed in row halves so the store overlaps the unpack.
    own_hw = ps[:, HOWN:HOWN + 8, 0:NDW].bitcast(U16)   # [P, 8, 64]
    outf = pool.tile([P, 8, W], F32, name="outf", tag="B")
    outf_r = outf.rearrange("p r (h k) -> p r h k", k=16)
    out_r = out_d.rearrange("(p r w) -> p r w", p=P, r=R)
    for hf, (r0, r1) in enumerate(((0, 4), (4, 8))):
        tub = pool.tile([P, 4, 16, 64], U16, name=f"tub{hf}",
                        tag="SC")
        for k in range(16):
            ve.tensor_single_scalar(tub[:, :, k, :], own_hw[:, r0:r1, :],
                                    15 - k, op=A.logical_shift_left)
        tub_px = tub.bitcast(I16).rearrange("p r k h -> p r h k")
        ve.tensor_single_scalar(outf_r[:, r0:r1, :, :], tub_px[:, :, :, :],
                                0, op=A.is_lt)
        nc.sync.dma_start(out=out_r[:, r0:r1, :], in_=outf[:, r0:r1, :])


_CACHE = {}


def _get_built():
    if "nc" not in _CACHE:
        from concourse import bacc
        nc = bacc.Bacc(None)
        img_d = nc.declare_dram_parameter("img", [H * W], F32, isOutput=False)
        out_d = nc.declare_dram_parameter("out", [H * W], F32, isOutput=True)
        with TileContext(nc) as tc:
            with tc.tile_pool(name="main", bufs=1) as pool:
                build_canny(nc, tc, pool, img_d, out_d)
        nc.finalize()
        _CACHE["nc"] = nc
    return _CACHE["nc"]


TRACE = False        # set True (e.g. from test.py) to capture an NTFF profile
LAST_RESULT = None   # BassKernelResults of the most recent run


def kernel(image):
    global LAST_RESULT
    image = np.ascontiguousarray(np.asarray(image), dtype=np.float32)
    B = image.shape[0]
    assert image.shape == (B, 1, H, W)
    nc = _get_built()
    in_maps = [{"img": image[i, 0].reshape(-1)} for i in range(B)]
    res = run_bass_kernel_spmd(nc, in_maps, core_ids=list(range(B)),
                               trace=TRACE)
    LAST_RESULT = res
    out = np.stack([r["out"].reshape(H, W) for r in res.results])
    return out[:, None].astype(np.float32)


# revision 38
# speedup vs baseline: 1.0038x; 1.0038x over previous
"""Canny edge detector on 8 Trainium2 NeuronCores — pure data-parallel (1 image/core).

Pipeline per core (image 1024x1024 f32):
  1. 5x5 Gaussian blur (separable: vertical then horizontal 5-tap, exact f32)
  2. Sobel gx, gy (separable 3-taps)
  3. NMS using squared magnitudes (no sqrt / atan2 needed)
  4. Hysteresis: 16 iterations of 3x3 binary dilation masked by weak, on
     bit-packed state (32 px/word) with per-row gutter words.

Layout: "multirow" — partition p holds image rows [8p+d] in its free
dimension, row pitch 1028 (2 zero gutter cols each side) so ALL 8-neighbor
shifts are free-dim AP offsets.  Vertical halos come from overlapping HBM
loads (img) and SBUF->SBUF DMA halo refreshes (blurred, msq, packed state).

Engine facts (BIR-verifier-probed): Pool/GPSIMD supports ONLY f32
tensor_tensor add/sub/mult, tensor_single_scalar mult/add/max/compares,
copy, memset, iota.  No STT, no TT max/compare, no bitwise/shifts.
DVE does everything; Act does single-input activations (copy-scale,
square, relu).  So:
  - TT add/sub/mult ops column-split DVE|Pool at 672 (rates 1.042 vs 1.984)
  - single-scalar ops split at 745 (0.521 vs 1.389)
  - fused STT combines (a*s + b) run on DVE for cols [0:782], decomposed
    TSS-mult + TT-add on Pool for the rest
  - NMS pair maxes: DVE TT-max [0:810], Pool sub + Act relu + Pool add
    beyond (max(a,b) = b + relu(a-b); <=1-ulp rounding, flips only exact
    NMS ties — probability ~1e-12 per pixel)
  - hysteresis bitwise loop is DVE-only (hardware restriction), with
    interior-first iterations after each halo exchange to hide DMA latency
"""
import numpy as np

import concourse.bass as bass
import concourse.mybir as mybir
from concourse.tile import TileContext
from concourse.bass_utils import run_bass_kernel_spmd

P = 128          # partitions
R = 8            # image rows per partition
H = W = 1024
RP = 1028        # row pitch (2 gutter cols + 1024 data + 2 gutter cols)
DOF = 2          # data column offset within a row slot

# packed layout: 32 px/word -> 32 data words + 1 zero gutter word per row
PW = 33
NDW = 32

# hysteresis packed tile: 1 margin + (J halo + 8 own + J halo) data rows + 1 margin
HJ = 2           # halo rows == refresh cadence (iterations between halo refreshes)
HNR = 2 + 8 + 2 * HJ
HD0 = 1          # first data row (halo-top) in packed tiles
HOWN = 1 + HJ    # first own row in packed tiles

F32 = mybir.dt.float32
U32 = mybir.dt.uint32
U16 = mybir.dt.uint16
I16 = mybir.dt.int16
I32 = mybir.dt.int32
I8 = mybir.dt.int8

# DVE | Pool column splits (DVE gets [0:r), Pool [r:W))
RA = 672     # TT add/sub/mult      (DVE 1.042 vs Pool 1.984 ns/elem)
RS = 745     # single-scalar ops    (DVE 0.521 vs Pool 1.389)
RC = 782     # STT combine vs Pool TSS-mult + TT-add  (1.042 vs 3.373)
RX = 810     # TT max vs Pool sub + Act relu + Pool add (1.042 vs 3.968)
RN = 840     # STT cmp vs Pool TSS-mult + TT-sub + TSS-cmp (1.042 vs 4.762)
RT = 390     # pack-tree level-1 split of 512 pair sums


def _f32_consts():
    ax = np.arange(5, dtype=np.float32) - np.float32(2.0)
    g = np.exp(-(ax ** 2) / np.float32(2.0)).astype(np.float32)
    g = (g / g.sum()).astype(np.float32)
    c1 = np.float32(np.tan(np.deg2rad(22.5)) ** 2)
    c2 = np.float32(np.tan(np.deg2rad(67.5)) ** 2)

    def sqrt_thresh(t):
        t = np.float32(t)
        x = np.float32(t) * np.float32(t)
        while np.sqrt(np.float32(x)) >= t:
            x = np.nextafter(x, np.float32(0.0), dtype=np.float32)
        while np.sqrt(np.float32(x)) < t:
            x = np.nextafter(x, np.float32(np.inf), dtype=np.float32)
        return np.float32(x)

    return g, c1, c2, sqrt_thresh(0.1), sqrt_thresh(0.2)


def build_canny(nc, tc, pool, img_d, out_d, stage=99):
    import os
    stage = int(os.environ.get("CANNY_STAGE", stage))
    from concourse.alu_op_type import AluOpType as A
    g, c1, c2, tlow, thigh = _f32_consts()
    ve = nc.vector
    gp = nc.gpsimd
    se = nc.scalar

    def bail():
        z = pool.tile([P, 8, W], F32, name="zz", tag="C")
        ve.memset(z[:, :, :], 0.0)
        nc.sync.dma_start(out=out_d.rearrange("(p r w) -> p r w", p=P, r=R),
                          in_=z[:, :, :])

    def sp(r):
        return ((ve, 0, r), (gp, r, W))

    def zero_gutters(eng, t, nr):
        eng.memset(t[:, 0:nr, 0:DOF], 0.0)
        eng.memset(t[:, 0:nr, DOF + W:RP], 0.0)

    def comb(mk_dst, mk_src, s):
        """dst += s*src: DVE fused STT on [0:RC]; Pool scales src in place
        (src must be dead afterwards) then adds, on [RC:W]."""
        ve.scalar_tensor_tensor(mk_dst(0, RC), mk_src(0, RC), float(s),
                                mk_dst(0, RC), op0=A.mult, op1=A.add)
        gp.tensor_single_scalar(mk_src(RC, W), mk_src(RC, W), float(s),
                                op=A.mult)
        gp.tensor_tensor(mk_dst(RC, W), mk_src(RC, W), mk_dst(RC, W),
                         op=A.add)

    def comb_act(mk_dst, mk_src, s):
        """dst += s*src with the scale on Act (in place over src, which must
        be dead afterwards) and the add column-split DVE|Pool.  Same two
        roundings as the fused STT path — bit-identical."""
        se.mul(mk_src(0, W), mk_src(0, W), float(s))
        for eng, c0, c1_ in sp(RA):
            eng.tensor_tensor(mk_dst(c0, c1_), mk_src(c0, c1_),
                              mk_dst(c0, c1_), op=A.add)

    # per-partition integer scalar constants for bitwise scalar_tensor_tensor
    cst = pool.tile([P, 4], U32, name="cst", tag="tcst")
    ve.memset(cst[:, 0:1], 1)
    ve.memset(cst[:, 1:2], 16)
    ve.memset(cst[:, 2:3], 31)
    C1A, C16A, C31A = cst[:, 0:1], cst[:, 1:2], cst[:, 2:3]

    # ---------------- load image (rows 8p-2 .. 8p+10) ----------------
    img = pool.tile([P, 12, W], F32, name="img", tag="A")
    # out-of-image halo rows must be zero; the loads below overwrite all but
    # partition 0 / 127 edges (compute ops cannot start at partition 127, so
    # full-partition memsets, one per engine, before the loads)
    ve.memset(img[:, 0:2, :], 0.0)
    gp.memset(img[:, 10:12, :], 0.0)  # full partitions: p127 can't be sliced

    img_rows = img_d.rearrange("(n w) -> n w", w=W)
    # small partition-edge loads first, then the main window in column
    # halves (left half feeds the DVE shares of the first blur ops, so
    # compute starts before the right half lands)
    nc.sync.dma_start(out=img[0:1, 2:12, :],
                      in_=img_rows[0:10, :].rearrange("(p r) w -> p r w", p=1))
    nc.sync.dma_start(out=img[P - 1:P, 0:10, :],
                      in_=img_rows[H - 10:H, :].rearrange("(p r) w -> p r w", p=1))
    for c0, c1_ in ((0, RA), (RA, W)):
        img_win = bass.AP(img_d, (R - 2) * W + c0,
                          [[R * W, P - 2], [W, 12], [1, c1_ - c0]])
        nc.sync.dma_start(out=img[1:P - 1, :, c0:c1_], in_=img_win)

    # ---------------- vertical 5-tap blur -> blurv (own 8 rows) ----------------
    blurv = pool.tile([P, 8, RP], F32, name="blurv", tag="B")
    zero_gutters(ve, blurv, 8)
    pa1 = pool.tile([P, 8, W], F32, name="pa1", tag="C")
    pa2 = pool.tile([P, 8, W], F32, name="pa2", tag="F")
    for eng, c0, c1_ in sp(RA):
        eng.tensor_tensor(pa1[:, :, c0:c1_], img[:, 1:9, c0:c1_],
                          img[:, 3:11, c0:c1_], op=A.add)
        eng.tensor_tensor(pa2[:, :, c0:c1_], img[:, 0:8, c0:c1_],
                          img[:, 4:12, c0:c1_], op=A.add)
    se.mul(blurv[:, :, DOF:DOF + RA], img[:, 2:10, 0:RA], float(g[2]))
    se.mul(blurv[:, :, DOF + RA:DOF + W], img[:, 2:10, RA:W], float(g[2]))
    comb(lambda a, b: blurv[:, :, DOF + a:DOF + b],
         lambda a, b: pa1[:, :, a:b], g[1])
    comb(lambda a, b: blurv[:, :, DOF + a:DOF + b],
         lambda a, b: pa2[:, :, a:b], g[0])

    if stage <= 1:
        bail()
        return

    # ---------------- horizontal 5-tap blur -> blurred [10 rows, own at 1..9] ---
    blurred = pool.tile([P, 10, RP], F32, name="blurred", tag="A")
    pb1 = pool.tile([P, 8, W], F32, name="pb1", tag="C")
    pb2 = pool.tile([P, 8, W], F32, name="pb2", tag="F")
    for eng, c0, c1_ in sp(RA):
        eng.tensor_tensor(pb1[:, :, c0:c1_],
                          blurv[:, :, DOF + c0 - 1:DOF + c1_ - 1],
                          blurv[:, :, DOF + c0 + 1:DOF + c1_ + 1], op=A.add)
        eng.tensor_tensor(pb2[:, :, c0:c1_],
                          blurv[:, :, DOF + c0 - 2:DOF + c1_ - 2],
                          blurv[:, :, DOF + c0 + 2:DOF + c1_ + 2], op=A.add)
    se.mul(blurred[:, 1:9, DOF:DOF + W], blurv[:, :, DOF:DOF + W], float(g[2]))
    comb(lambda a, b: blurred[:, 1:9, DOF + a:DOF + b],
         lambda a, b: pb1[:, :, a:b], g[1])
    comb(lambda a, b: blurred[:, 1:9, DOF + a:DOF + b],
         lambda a, b: pb2[:, :, a:b], g[0])
    # halo refresh: row 0 <- p-1 own row 7 (tile row 8); row 9 <- p+1 own row 0
    ve.memset(blurred[:, 0:1, :], 0.0)
    ve.memset(blurred[:, 9:10, :], 0.0)
    nc.sync.dma_start(out=blurred[1:P, 0:1, DOF:DOF + W],
                      in_=blurred[0:P - 1, 8:9, DOF:DOF + W])
    nc.scalar.dma_start(out=blurred[0:P - 1, 9:10, DOF:DOF + W],
                        in_=blurred[1:P, 1:2, DOF:DOF + W])

    if stage <= 2:
        bail()
        return

    # ---------------- sobel vertical parts (own 8 rows) ----------------
    # wx = bl[r-1] + 2 bl[r] + bl[r+1] ; vy = bl[r+1] - bl[r-1]
    wx = pool.tile([P, 8, RP], F32, name="wx", tag="C")
    vy = pool.tile([P, 8, RP], F32, name="vy", tag="F")
    zero_gutters(ve, wx, 8)
    zero_gutters(gp, vy, 8)
    for eng, c0, c1_ in sp(RA):
        eng.tensor_tensor(wx[:, :, DOF + c0:DOF + c1_],
                          blurred[:, 0:8, DOF + c0:DOF + c1_],
                          blurred[:, 2:10, DOF + c0:DOF + c1_], op=A.add)
        eng.tensor_tensor(vy[:, :, DOF + c0:DOF + c1_],
                          blurred[:, 2:10, DOF + c0:DOF + c1_],
                          blurred[:, 0:8, DOF + c0:DOF + c1_], op=A.subtract)
    # wx += 2*bl(center); Pool side scales blurred rows 1:9 in place (dead after)
    comb(lambda a, b: wx[:, :, DOF + a:DOF + b],
         lambda a, b: blurred[:, 1:9, DOF + a:DOF + b], 2.0)

    # ---------------- sobel horizontal parts ----------------
    gx = pool.tile([P, 8, RP], F32, name="gx", tag="B")
    gy = pool.tile([P, 8, RP], F32, name="gy", tag="A")
    gx_d = gx[:, :, DOF:DOF + W]
    gy_d = gy[:, :, DOF:DOF + W]
    for eng, c0, c1_ in sp(RA):
        eng.tensor_tensor(gx[:, :, DOF + c0:DOF + c1_],
                          wx[:, :, DOF + c0 + 1:DOF + c1_ + 1],
                          wx[:, :, DOF + c0 - 1:DOF + c1_ - 1], op=A.subtract)
        eng.tensor_tensor(gy[:, :, DOF + c0:DOF + c1_],
                          vy[:, :, DOF + c0 - 1:DOF + c1_ - 1],
                          vy[:, :, DOF + c0 + 1:DOF + c1_ + 1], op=A.add)
    # gy += 2*vy; Pool side scales vy in place (dead after)
    comb(lambda a, b: gy[:, :, DOF + a:DOF + b],
         lambda a, b: vy[:, :, DOF + a:DOF + b], 2.0)

    if stage <= 3:
        bail()
        return

    # ---------------- sign of gx*gy, squares, msq ----------------
    # diagonal-class mask: sm = (gx*gy < 0).  Product underflow to +-0 only
    # happens when msq is far below the weak threshold, where the NMS
    # direction choice can't affect the output.
    smw = pool.tile([P, 8, W], F32, name="smw", tag="C")
    sm = pool.tile([P, 8, W], I8, name="sm", tag="G2")
    for eng, c0, c1_ in sp(RA):
        eng.tensor_tensor(smw[:, :, c0:c1_], gx[:, :, DOF + c0:DOF + c1_],
                          gy[:, :, DOF + c0:DOF + c1_], op=A.mult)
    for eng, c0, c1_ in sp(RS):
        eng.tensor_single_scalar(sm[:, :, c0:c1_], smw[:, :, c0:c1_], 0.0,
                                 op=A.is_lt)

    se.square(gx_d, gx_d)   # sqx
    se.square(gy_d, gy_d)   # sqy
    sqx, sqy = gx, gy
    sqx_d, sqy_d = gx_d, gy_d

    # msq [10 rows, own at 1..9] with DMA halo refresh (before nb0/nb2 so the
    # Pool decompositions may clobber sqx afterwards)
    msq = pool.tile([P, 10, RP], F32, name="msq", tag="F")
    zero_gutters(ve, msq, 10)
    for eng, c0, c1_ in sp(RA):
        eng.tensor_tensor(msq[:, 1:9, DOF + c0:DOF + c1_],
                          sqx[:, :, DOF + c0:DOF + c1_],
                          sqy[:, :, DOF + c0:DOF + c1_], op=A.add)
    ve.memset(msq[:, 0:1, :], 0.0)
    ve.memset(msq[:, 9:10, :], 0.0)
    nc.sync.dma_start(out=msq[1:P, 0:1, :], in_=msq[0:P - 1, 8:9, :])
    nc.scalar.dma_start(out=msq[0:P - 1, 9:10, :], in_=msq[1:P, 1:2, :])

    # direction classes (int8 0/1): nb0 = sqy < c1*sqx ; nb2 = sqy >= c2*sqx
    # DVE: fused STT on [0:RN].  Pool on [RN:W]: t = c*sqx; d = t - sqy;
    # mask = sign test (exact: f32 subtract has exact sign).
    nb0 = pool.tile([P, 8, W], I8, name="nb0", tag="G")
    nb2 = pool.tile([P, 8, W], I8, name="nb2", tag="Hh")
    sc = pool.tile([P, 8, W - RN], F32, name="sc", tag="SC",
                   padded_shape=[P, 8, W - RX])
    ve.scalar_tensor_tensor(nb0[:, :, 0:RN], sqx_d[:, :, 0:RN], float(c1),
                            sqy_d[:, :, 0:RN], op0=A.mult, op1=A.is_gt)
    ve.scalar_tensor_tensor(nb2[:, :, 0:RN], sqx_d[:, :, 0:RN], float(c2),
                            sqy_d[:, :, 0:RN], op0=A.mult, op1=A.is_le)
    gp.tensor_single_scalar(sc[:, :, :], sqx_d[:, :, RN:W], float(c1),
                            op=A.mult)
    gp.tensor_tensor(sc[:, :, :], sc[:, :, :], sqy_d[:, :, RN:W],
                     op=A.subtract)
    gp.tensor_single_scalar(nb0[:, :, RN:W], sc[:, :, :], 0.0, op=A.is_gt)
    # nb2 Pool side: clobber sqx in place (dead after this)
    gp.tensor_single_scalar(sqx_d[:, :, RN:W], sqx_d[:, :, RN:W], float(c2),
                            op=A.mult)
    gp.tensor_tensor(sqx_d[:, :, RN:W], sqx_d[:, :, RN:W],
                     sqy_d[:, :, RN:W], op=A.subtract)
    gp.tensor_single_scalar(nb2[:, :, RN:W], sqx_d[:, :, RN:W], 0.0,
                            op=A.is_le)

    if stage <= 4:
        bail()
        return

    # ---------------- NMS: directional pair maxes + predicated select ----------
    def msq_sh(dr, dj, c0=0, c1_=W):
        return msq[:, 1 + dr:9 + dr, DOF + dj + c0:DOF + dj + c1_]

    # Pair maxes all on DVE (Pool cannot do TT max, and decomposed
    # sub/relu/add chains stall the pred cascade).  While DVE runs the maxes
    # and the predicated-copy cascade, Pool packs the threshold planes
    # tw/ts = (msq >= t) into 16-bit halfword sums via the pairwise doubling
    # tree (TSS-mult + TT-add, Pool-legal; Act takes the odd-scales).
    # Packing distributes over AND, so afterwards DVE only packs
    # kb = (msq >= M) and bitwise-ANDs the packed words:
    #   pack(kb AND t) = pack(kb) & pack(t)       (bits are independent)
    M = pool.tile([P, 8, W], F32, name="M", tag="B")
    m_d2 = pool.tile([P, 8, W], F32, name="m_d2", tag="A")
    m_ns = pool.tile([P, 8, W], F32, name="m_ns", tag="C")

    # packed halfword-sum planes (f32 sums 0..65535, exact)
    ptw = pool.tile([P, 8, 64], F32, name="ptw", tag="PTW")
    pts = pool.tile([P, 8, 64], F32, name="pts", tag="PTS")

    def pool_pack_tw():
        # quarter-column passes, entirely on Pool (TSS in-place odd-scales +
        # TT adds) so the tree never waits on another engine.
        mults = (2.0, 4.0, 16.0, 256.0)
        for q in range(4):
            th = pool.tile([P, 8, 256], F32, name="th", tag="SC")
            gp.tensor_single_scalar(th[:, :, :],
                                    msq_sh(0, 0, 256 * q, 256 * q + 256),
                                    float(tlow), op=A.is_ge)
            cur = th
            for li in range(4):
                n = 128 >> li
                cur_r = cur.rearrange("p r (j two) -> p r j two", two=2)
                gp.tensor_single_scalar(cur_r[:, :, :, 1], cur_r[:, :, :, 1],
                                        mults[li], op=A.mult)
                if li < 3:
                    nxt = pool.tile([P, 8, n], F32, name=f"pt{li}",
                                    tag=("SD", "SE", "SD")[li])
                    gp.tensor_tensor(nxt[:, :, :], cur_r[:, :, :, 1],
                                     cur_r[:, :, :, 0], op=A.add)
                    cur = nxt
                else:
                    gp.tensor_tensor(ptw[:, :, 16 * q:16 * q + 16],
                                     cur_r[:, :, :, 1], cur_r[:, :, :, 0],
                                     op=A.add)

    def pmax_dve(dst, a_dr, a_dj, b_dr, b_dj):
        ve.tensor_tensor(dst[:, :, :], msq_sh(a_dr, a_dj),
                         msq_sh(b_dr, b_dj), op=A.max)

    pool_pack_tw()
    pmax_dve(m_d2, -1, -1, 1, 1)
    pmax_dve(M, -1, 1, 1, -1)
    HALF = 512
    for c0, c1_ in ((0, HALF), (HALF, W)):
        ve.copy_predicated(M[:, :, c0:c1_], sm[:, :, c0:c1_],
                           m_d2[:, :, c0:c1_])
    m_ew = pool.tile([P, 8, W], F32, name="m_ew", tag="A")
    pmax_dve(m_ns, -1, 0, 1, 0)
    for c0, c1_ in ((0, HALF), (HALF, W)):
        ve.copy_predicated(M[:, :, c0:c1_], nb2[:, :, c0:c1_],
                           m_ns[:, :, c0:c1_])
    pmax_dve(m_ew, 0, 1, 0, -1)
    for c0, c1_ in ((0, HALF), (HALF, W)):
        ve.copy_predicated(M[:, :, c0:c1_], nb0[:, :, c0:c1_],
                           m_ew[:, :, c0:c1_])

    if stage <= 5:
        bail()
        return

    # ---------------- keep-plane pack + AND with threshold planes -------------
    ps = pool.tile([P, HNR, PW], U32, name="ps", tag="tps")
    pw_ = pool.tile([P, HNR, PW], U32, name="pw_", tag="tpw")
    gp.memset(ps[:, :, :], 0)
    gp.memset(pw_[:, :, :], 0)

    # kb = (msq >= M) via exact-sign subtract
    kb = pool.tile([P, 8, W], F32, name="kb", tag="A")
    for eng, c0, c1_ in sp(RA):
        eng.tensor_tensor(kb[:, :, c0:c1_], msq_sh(0, 0, c0, c1_),
                          M[:, :, c0:c1_], op=A.subtract)
    for eng, c0, c1_ in sp(RS):
        eng.tensor_single_scalar(kb[:, :, c0:c1_], kb[:, :, c0:c1_], 0.0,
                                 op=A.is_ge)

    # DVE trees (fused STT levels, per column half with small scratch):
    # one on kb, one on the strong-threshold plane ts
    def dve_tree(plane, out_plane):
        mults = (2.0, 4.0, 16.0, 256.0)
        for hh in (0, 1):
            cur = plane[:, :, 512 * hh:512 * hh + 512]
            n = 256
            tags = ("SC", "SD", "SC")
            for li in range(4):
                cur_r = cur.rearrange("p r (j two) -> p r j two", two=2)
                nxt = (pool.tile([P, 8, n], F32, name=f"kt{li}",
                                 tag=tags[li])
                       if li < 3 else out_plane[:, :, 32 * hh:32 * hh + 32])
                ve.scalar_tensor_tensor(nxt, cur_r[:, :, :, 1], mults[li],
                                        cur_r[:, :, :, 0],
                                        op0=A.mult, op1=A.add)
                cur = nxt
                n //= 2

    ts = pool.tile([P, 8, W], F32, name="ts", tag="B")
    for eng, c0, c1_ in sp(RS):
        eng.tensor_single_scalar(ts[:, :, c0:c1_], msq_sh(0, 0, c0, c1_),
                                 float(thigh), op=A.is_ge)
    pkb = pool.tile([P, 8, 64], F32, name="pkb", tag="PKB")
    dve_tree(kb, pkb)
    dve_tree(ts, pts)

    # convert the three halfword-sum planes to u32 and AND packed words
    ikb = pool.tile([P, 8, 64], U32, name="ikb", tag="SC")
    itw = pool.tile([P, 8, 64], U32, name="itw", tag="SD")
    its = pool.tile([P, 8, 64], U32, name="its", tag="SE")
    ve.tensor_copy(ikb[:, :, :], pkb[:, :, :])
    ve.tensor_copy(itw[:, :, :], ptw[:, :, :])
    ve.tensor_copy(its[:, :, :], pts[:, :, :])
    ve.tensor_tensor(itw[:, :, :], itw[:, :, :], ikb[:, :, :],
                     op=A.bitwise_and)
    ve.tensor_tensor(its[:, :, :], its[:, :, :], ikb[:, :, :],
                     op=A.bitwise_and)
    for t, srcw in ((pw_, itw), (ps, its)):
        hv = srcw.rearrange("p r (s two) -> p r s two", two=2)
        ve.scalar_tensor_tensor(t[:, HOWN:HOWN + 8, 0:NDW], hv[:, :, :, 1],
                                C16A, hv[:, :, :, 0],
                                op0=A.logical_shift_left, op1=A.bitwise_or)

    # ---------------- packed halos ----------------
    def refresh_halos(t):
        nc.sync.dma_start(out=t[1:P, HD0:HD0 + HJ, :],
                          in_=t[0:P - 1, HOWN + 8 - HJ:HOWN + 8, :])
        nc.scalar.dma_start(out=t[0:P - 1, HOWN + 8:HOWN + 8 + HJ, :],
                            in_=t[1:P, HOWN:HOWN + HJ, :])

    refresh_halos(pw_)
    refresh_halos(ps)

    if stage <= 6:
        bail()
        return

    # ---------------- 16 iterations of masked dilation (packed) --------------
    # Bitwise u32 ops are DVE-only on TRN2, so the whole packed loop runs on
    # the vector engine.  V has a zero gutter slot on each side so every
    # word-shift read stays in-tile.  Post-refresh iterations process
    # interior rows first so the halo-exchange DMA latency hides.
    nd = 8 + 2 * HJ
    Vd = pool.tile([P, HNR, NDW + 2], U32, name="Vd", tag="tV")  # gutter,0..31,gutter
    Hd = pool.tile([P, HNR, NDW], U32, name="Hd", tag="tH")
    ve.memset(Vd[:, :, :], 0)
    ve.memset(Hd[:, :, :], 0)

    def rsel(t, g, dr, w0, w1):
        # row-group selector: 'all' rows 1..12, 'core' 4..9, and the two
        # contiguous rim bands (STT ops only accept 2D/3D APs)
        if g == "all":
            return t[:, HD0 + dr:HD0 + nd + dr, w0:w1]
        if g == "core":
            return t[:, 4 + dr:10 + dr, w0:w1]
        if g == "rim1":
            return t[:, 1 + dr:4 + dr, w0:w1]
        return t[:, 10 + dr:13 + dr, w0:w1]

    def hyst_iter(groups=("all",)):
        for g in groups:
            V = rsel(Vd, g, 0, 1, NDW + 1)
            Hh = rsel(Hd, g, 0, 0, NDW)
            ve.tensor_tensor(V, rsel(ps, g, -1, 0, NDW),
                             rsel(ps, g, 1, 0, NDW), op=A.bitwise_or)
            ve.tensor_tensor(V, rsel(ps, g, 0, 0, NDW), V, op=A.bitwise_or)
            ve.scalar_tensor_tensor(Hh, V, C1A, V, op0=A.logical_shift_left,
                                    op1=A.bitwise_or)
            ve.scalar_tensor_tensor(Hh, V, C1A, Hh, op0=A.logical_shift_right,
                                    op1=A.bitwise_or)
            ve.scalar_tensor_tensor(Hh, rsel(Vd, g, 0, 0, NDW), C31A, Hh,
                                    op0=A.logical_shift_right, op1=A.bitwise_or)
            ve.scalar_tensor_tensor(Hh, rsel(Vd, g, 0, 2, NDW + 2), C31A, Hh,
                                    op0=A.logical_shift_left, op1=A.bitwise_or)
        for g in groups:
            ve.tensor_tensor(rsel(ps, g, 0, 0, NDW), rsel(Hd, g, 0, 0, NDW),
                             rsel(pw_, g, 0, 0, NDW), op=A.bitwise_and)

    n_iters = int(os.environ.get("CANNY_HYST_ITERS", 16))
    no_refresh = int(os.environ.get("CANNY_NO_REFRESH", 0))  # timing expt only
    hide = int(os.environ.get("CANNY_HIDE_REFRESH", 1))
    for it in range(n_iters):
        post_refresh = hide and it > 0 and it % HJ == 0 and not no_refresh
        hyst_iter(("core", "rim1", "rim2") if post_refresh else ("all",))
        if (it + 1) % HJ == 0 and it < n_iters - 1 and not no_refresh:
            refresh_halos(ps)

    if stage <= 7:
        bail()
        return

    # ---------------- unpack own rows -> f32 0/1 and store --------------------
    # ps own words viewed as u16 halfwords h (= pixels 16h..16h+15).  For
    # each bit position k: tub[k][h] = hw[h] << (15-k) (single-src imm-shift
    # TSS on packed u16 = 4x DVE mode), then sign test in the transposed
    # view.  Processed in row halves so the store overlaps the unpack.
    own_hw = ps[:, HOWN:HOWN + 8, 0:NDW].bitcast(U16)   # [P, 8, 64]
    outf = pool.tile([P, 8, W], F32, name="outf", tag="B")
    outf_r = outf.rearrange("p r (h k) -> p r h k", k=16)
    out_r = out_d.rearrange("(p r w) -> p r w", p=P, r=R)
    for hf, (r0, r1) in enumerate(((0, 4), (4, 8))):
        tub = pool.tile([P, 4, 16, 64], U16, name=f"tub{hf}",
                        tag="SC")
        for k in range(16):
            ve.tensor_single_scalar(tub[:, :, k, :], own_hw[:, r0:r1, :],
                                    15 - k, op=A.logical_shift_left)
        tub_px = tub.bitcast(I16).rearrange("p r k h -> p r h k")
        ve.tensor_single_scalar(outf_r[:, r0:r1, :, :], tub_px[:, :, :, :],
                                0, op=A.is_lt)
        nc.sync.dma_start(out=out_r[:, r0:r1, :], in_=outf[:, r0:r1, :])


_CACHE = {}


def _get_built():
    if "nc" not in _CACHE:
        from concourse import bacc
        nc = bacc.Bacc(None)
        img_d = nc.declare_dram_parameter("img", [H * W], F32, isOutput=False)
        out_d = nc.declare_dram_parameter("out", [H * W], F32, isOutput=True)
        with TileContext(nc) as tc:
            with tc.tile_pool(name="main", bufs=1) as pool:
                build_canny(nc, tc, pool, img_d, out_d)
        nc.finalize()
        _CACHE["nc"] = nc
    return _CACHE["nc"]


TRACE = False        # set True (e.g. from test.py) to capture an NTFF profile
LAST_RESULT = None   # BassKernelResults of the most recent run


def kernel(image):
    global LAST_RESULT
    image = np.ascontiguousarray(np.asarray(image), dtype=np.float32)
    B = image.shape[0]
    assert image.shape == (B, 1, H, W)
    nc = _get_built()
    in_maps = [{"img": image[i, 0].reshape(-1)} for i in range(B)]
    res = run_bass_kernel_spmd(nc, in_maps, core_ids=list(range(B)),
                               trace=TRACE)
    LAST_RESULT = res
    out = np.stack([r["out"].reshape(H, W) for r in res.results])
    return out[:, None].astype(np.float32)


# revision 45
# speedup vs baseline: 1.0075x; 1.0037x over previous
"""Canny edge detector on 8 Trainium2 NeuronCores — pure data-parallel (1 image/core).

Pipeline per core (image 1024x1024 f32):
  1. 5x5 Gaussian blur (separable: vertical then horizontal 5-tap, exact f32)
  2. Sobel gx, gy (separable 3-taps)
  3. NMS using squared magnitudes (no sqrt / atan2 needed)
  4. Hysteresis: 16 iterations of 3x3 binary dilation masked by weak, on
     bit-packed state (32 px/word) with per-row gutter words.

Layout: "multirow" — partition p holds image rows [8p+d] in its free
dimension, row pitch 1028 (2 zero gutter cols each side) so ALL 8-neighbor
shifts are free-dim AP offsets.  Vertical halos come from overlapping HBM
loads (img) and SBUF->SBUF DMA halo refreshes (blurred, msq, packed state).

Engine facts (BIR-verifier-probed): Pool/GPSIMD supports ONLY f32
tensor_tensor add/sub/mult, tensor_single_scalar mult/add/max/compares,
copy, memset, iota.  No STT, no TT max/compare, no bitwise/shifts.
DVE does everything; Act does single-input activations (copy-scale,
square, relu).  So:
  - TT add/sub/mult ops column-split DVE|Pool at 672 (rates 1.042 vs 1.984)
  - single-scalar ops split at 745 (0.521 vs 1.389)
  - fused STT combines (a*s + b) run on DVE for cols [0:782], decomposed
    TSS-mult + TT-add on Pool for the rest
  - NMS pair maxes: DVE TT-max [0:810], Pool sub + Act relu + Pool add
    beyond (max(a,b) = b + relu(a-b); <=1-ulp rounding, flips only exact
    NMS ties — probability ~1e-12 per pixel)
  - hysteresis bitwise loop is DVE-only (hardware restriction), with
    interior-first iterations after each halo exchange to hide DMA latency
"""
import numpy as np

import concourse.bass as bass
import concourse.mybir as mybir
from concourse.tile import TileContext
from concourse.bass_utils import run_bass_kernel_spmd

P = 128          # partitions
R = 8            # image rows per partition
H = W = 1024
RP = 1028        # row pitch (2 gutter cols + 1024 data + 2 gutter cols)
DOF = 2          # data column offset within a row slot

# packed layout: 32 px/word -> 32 data words + 1 zero gutter word per row
PW = 33
NDW = 32

# hysteresis packed tile: 1 margin + (J halo + 8 own + J halo) data rows + 1 margin
HJ = 2           # halo rows == refresh cadence (iterations between halo refreshes)
HNR = 2 + 8 + 2 * HJ
HD0 = 1          # first data row (halo-top) in packed tiles
HOWN = 1 + HJ    # first own row in packed tiles

F32 = mybir.dt.float32
U32 = mybir.dt.uint32
U16 = mybir.dt.uint16
I16 = mybir.dt.int16
I32 = mybir.dt.int32
I8 = mybir.dt.int8

# DVE | Pool column splits (DVE gets [0:r), Pool [r:W))
RA = 672     # TT add/sub/mult      (DVE 1.042 vs Pool 1.984 ns/elem)
RS = 745     # single-scalar ops    (DVE 0.521 vs Pool 1.389)
RC = 782     # STT combine vs Pool TSS-mult + TT-add  (1.042 vs 3.373)
RX = 810     # TT max vs Pool sub + Act relu + Pool add (1.042 vs 3.968)
RN = 840     # STT cmp vs Pool TSS-mult + TT-sub + TSS-cmp (1.042 vs 4.762)
RT = 390     # pack-tree level-1 split of 512 pair sums


def _f32_consts():
    ax = np.arange(5, dtype=np.float32) - np.float32(2.0)
    g = np.exp(-(ax ** 2) / np.float32(2.0)).astype(np.float32)
    g = (g / g.sum()).astype(np.float32)
    c1 = np.float32(np.tan(np.deg2rad(22.5)) ** 2)
    c2 = np.float32(np.tan(np.deg2rad(67.5)) ** 2)

    def sqrt_thresh(t):
        t = np.float32(t)
        x = np.float32(t) * np.float32(t)
        while np.sqrt(np.float32(x)) >= t:
            x = np.nextafter(x, np.float32(0.0), dtype=np.float32)
        while np.sqrt(np.float32(x)) < t:
            x = np.nextafter(x, np.float32(np.inf), dtype=np.float32)
        return np.float32(x)

    return g, c1, c2, sqrt_thresh(0.1), sqrt_thresh(0.2)


def build_canny(nc, tc, pool, img_d, out_d, stage=99):
    import os
    stage = int(os.environ.get("CANNY_STAGE", stage))
    from concourse.alu_op_type import AluOpType as A
    g, c1, c2, tlow, thigh = _f32_consts()
    ve = nc.vector
    gp = nc.gpsimd
    se = nc.scalar

    def bail():
        z = pool.tile([P, 8, W], F32, name="zz", tag="C")
        ve.memset(z[:, :, :], 0.0)
        nc.sync.dma_start(out=out_d.rearrange("(p r w) -> p r w", p=P, r=R),
                          in_=z[:, :, :])

    def sp(r):
        return ((ve, 0, r), (gp, r, W))

    def zero_gutters(eng, t, nr):
        eng.memset(t[:, 0:nr, 0:DOF], 0.0)
        eng.memset(t[:, 0:nr, DOF + W:RP], 0.0)

    def comb(mk_dst, mk_src, s):
        """dst += s*src: DVE fused STT on [0:RC]; Pool scales src in place
        (src must be dead afterwards) then adds, on [RC:W]."""
        ve.scalar_tensor_tensor(mk_dst(0, RC), mk_src(0, RC), float(s),
                                mk_dst(0, RC), op0=A.mult, op1=A.add)
        gp.tensor_single_scalar(mk_src(RC, W), mk_src(RC, W), float(s),
                                op=A.mult)
        gp.tensor_tensor(mk_dst(RC, W), mk_src(RC, W), mk_dst(RC, W),
                         op=A.add)

    def comb_act(mk_dst, mk_src, s):
        """dst += s*src with the scale on Act (in place over src, which must
        be dead afterwards) and the add column-split DVE|Pool.  Same two
        roundings as the fused STT path — bit-identical."""
        se.mul(mk_src(0, W), mk_src(0, W), float(s))
        for eng, c0, c1_ in sp(RA):
            eng.tensor_tensor(mk_dst(c0, c1_), mk_src(c0, c1_),
                              mk_dst(c0, c1_), op=A.add)

    # per-partition integer scalar constants for bitwise scalar_tensor_tensor
    cst = pool.tile([P, 4], U32, name="cst", tag="tcst")
    ve.memset(cst[:, 0:1], 1)
    ve.memset(cst[:, 1:2], 16)
    ve.memset(cst[:, 2:3], 31)
    C1A, C16A, C31A = cst[:, 0:1], cst[:, 1:2], cst[:, 2:3]

    # ---------------- load image (rows 8p-2 .. 8p+10) ----------------
    img = pool.tile([P, 12, W], F32, name="img", tag="A")
    # out-of-image halo rows must be zero; the loads below overwrite all but
    # partition 0 / 127 edges (compute ops cannot start at partition 127, so
    # full-partition memsets, one per engine, before the loads)
    ve.memset(img[:, 0:2, :], 0.0)
    gp.memset(img[:, 10:12, :], 0.0)  # full partitions: p127 can't be sliced

    img_rows = img_d.rearrange("(n w) -> n w", w=W)
    # small partition-edge loads first, then the main window in column
    # halves (left half feeds the DVE shares of the first blur ops, so
    # compute starts before the right half lands)
    nc.sync.dma_start(out=img[0:1, 2:12, :],
                      in_=img_rows[0:10, :].rearrange("(p r) w -> p r w", p=1))
    nc.sync.dma_start(out=img[P - 1:P, 0:10, :],
                      in_=img_rows[H - 10:H, :].rearrange("(p r) w -> p r w", p=1))
    for c0, c1_ in ((0, RA), (RA, W)):
        img_win = bass.AP(img_d, (R - 2) * W + c0,
                          [[R * W, P - 2], [W, 12], [1, c1_ - c0]])
        nc.sync.dma_start(out=img[1:P - 1, :, c0:c1_], in_=img_win)

    # ---------------- vertical 5-tap blur -> blurv (own 8 rows) ----------------
    blurv = pool.tile([P, 8, RP], F32, name="blurv", tag="B")
    zero_gutters(ve, blurv, 8)
    pa1 = pool.tile([P, 8, W], F32, name="pa1", tag="C")
    pa2 = pool.tile([P, 8, W], F32, name="pa2", tag="F")
    for eng, c0, c1_ in sp(RA):
        eng.tensor_tensor(pa1[:, :, c0:c1_], img[:, 1:9, c0:c1_],
                          img[:, 3:11, c0:c1_], op=A.add)
        eng.tensor_tensor(pa2[:, :, c0:c1_], img[:, 0:8, c0:c1_],
                          img[:, 4:12, c0:c1_], op=A.add)
    se.mul(blurv[:, :, DOF:DOF + RA], img[:, 2:10, 0:RA], float(g[2]))
    se.mul(blurv[:, :, DOF + RA:DOF + W], img[:, 2:10, RA:W], float(g[2]))
    comb(lambda a, b: blurv[:, :, DOF + a:DOF + b],
         lambda a, b: pa1[:, :, a:b], g[1])
    comb(lambda a, b: blurv[:, :, DOF + a:DOF + b],
         lambda a, b: pa2[:, :, a:b], g[0])

    if stage <= 1:
        bail()
        return

    # ---------------- horizontal 5-tap blur -> blurred [10 rows, own at 1..9] ---
    blurred = pool.tile([P, 10, RP], F32, name="blurred", tag="A")
    pb1 = pool.tile([P, 8, W], F32, name="pb1", tag="C")
    pb2 = pool.tile([P, 8, W], F32, name="pb2", tag="F")
    for eng, c0, c1_ in sp(RA):
        eng.tensor_tensor(pb1[:, :, c0:c1_],
                          blurv[:, :, DOF + c0 - 1:DOF + c1_ - 1],
                          blurv[:, :, DOF + c0 + 1:DOF + c1_ + 1], op=A.add)
        eng.tensor_tensor(pb2[:, :, c0:c1_],
                          blurv[:, :, DOF + c0 - 2:DOF + c1_ - 2],
                          blurv[:, :, DOF + c0 + 2:DOF + c1_ + 2], op=A.add)
    se.mul(blurred[:, 1:9, DOF:DOF + W], blurv[:, :, DOF:DOF + W], float(g[2]))
    comb(lambda a, b: blurred[:, 1:9, DOF + a:DOF + b],
         lambda a, b: pb1[:, :, a:b], g[1])
    comb(lambda a, b: blurred[:, 1:9, DOF + a:DOF + b],
         lambda a, b: pb2[:, :, a:b], g[0])
    # halo refresh: row 0 <- p-1 own row 7 (tile row 8); row 9 <- p+1 own row 0
    ve.memset(blurred[:, 0:1, :], 0.0)
    ve.memset(blurred[:, 9:10, :], 0.0)
    nc.sync.dma_start(out=blurred[1:P, 0:1, DOF:DOF + W],
                      in_=blurred[0:P - 1, 8:9, DOF:DOF + W])
    nc.scalar.dma_start(out=blurred[0:P - 1, 9:10, DOF:DOF + W],
                        in_=blurred[1:P, 1:2, DOF:DOF + W])

    if stage <= 2:
        bail()
        return

    # ---------------- sobel vertical parts (own 8 rows) ----------------
    # wx = bl[r-1] + 2 bl[r] + bl[r+1] ; vy = bl[r+1] - bl[r-1]
    wx = pool.tile([P, 8, RP], F32, name="wx", tag="C")
    vy = pool.tile([P, 8, RP], F32, name="vy", tag="F")
    zero_gutters(ve, wx, 8)
    zero_gutters(gp, vy, 8)
    for eng, c0, c1_ in sp(RA):
        eng.tensor_tensor(wx[:, :, DOF + c0:DOF + c1_],
                          blurred[:, 0:8, DOF + c0:DOF + c1_],
                          blurred[:, 2:10, DOF + c0:DOF + c1_], op=A.add)
        eng.tensor_tensor(vy[:, :, DOF + c0:DOF + c1_],
                          blurred[:, 2:10, DOF + c0:DOF + c1_],
                          blurred[:, 0:8, DOF + c0:DOF + c1_], op=A.subtract)
    # wx += 2*bl(center); Pool side scales blurred rows 1:9 in place (dead after)
    comb(lambda a, b: wx[:, :, DOF + a:DOF + b],
         lambda a, b: blurred[:, 1:9, DOF + a:DOF + b], 2.0)

    # ---------------- sobel horizontal parts ----------------
    gx = pool.tile([P, 8, RP], F32, name="gx", tag="B")
    gy = pool.tile([P, 8, RP], F32, name="gy", tag="A")
    gx_d = gx[:, :, DOF:DOF + W]
    gy_d = gy[:, :, DOF:DOF + W]
    for eng, c0, c1_ in sp(RA):
        eng.tensor_tensor(gx[:, :, DOF + c0:DOF + c1_],
                          wx[:, :, DOF + c0 + 1:DOF + c1_ + 1],
                          wx[:, :, DOF + c0 - 1:DOF + c1_ - 1], op=A.subtract)
        eng.tensor_tensor(gy[:, :, DOF + c0:DOF + c1_],
                          vy[:, :, DOF + c0 - 1:DOF + c1_ - 1],
                          vy[:, :, DOF + c0 + 1:DOF + c1_ + 1], op=A.add)
    # gy += 2*vy; Pool side scales vy in place (dead after)
    comb(lambda a, b: gy[:, :, DOF + a:DOF + b],
         lambda a, b: vy[:, :, DOF + a:DOF + b], 2.0)

    if stage <= 3:
        bail()
        return

    # ---------------- sign of gx*gy, squares, msq ----------------
    # diagonal-class mask: sm = (gx*gy < 0).  Product underflow to +-0 only
    # happens when msq is far below the weak threshold, where the NMS
    # direction choice can't affect the output.
    smw = pool.tile([P, 8, W], F32, name="smw", tag="C")
    sm = pool.tile([P, 8, W], I8, name="sm", tag="G2")
    for eng, c0, c1_ in sp(RA):
        eng.tensor_tensor(smw[:, :, c0:c1_], gx[:, :, DOF + c0:DOF + c1_],
                          gy[:, :, DOF + c0:DOF + c1_], op=A.mult)
    for eng, c0, c1_ in sp(RS):
        eng.tensor_single_scalar(sm[:, :, c0:c1_], smw[:, :, c0:c1_], 0.0,
                                 op=A.is_lt)

    se.square(gx_d, gx_d)   # sqx
    se.square(gy_d, gy_d)   # sqy
    sqx, sqy = gx, gy
    sqx_d, sqy_d = gx_d, gy_d

    # msq [10 rows, own at 1..9] with DMA halo refresh (before nb0/nb2 so the
    # Pool decompositions may clobber sqx afterwards)
    msq = pool.tile([P, 10, RP], F32, name="msq", tag="F")
    zero_gutters(ve, msq, 10)
    for eng, c0, c1_ in sp(RA):
        eng.tensor_tensor(msq[:, 1:9, DOF + c0:DOF + c1_],
                          sqx[:, :, DOF + c0:DOF + c1_],
                          sqy[:, :, DOF + c0:DOF + c1_], op=A.add)
    ve.memset(msq[:, 0:1, :], 0.0)
    ve.memset(msq[:, 9:10, :], 0.0)
    nc.sync.dma_start(out=msq[1:P, 0:1, :], in_=msq[0:P - 1, 8:9, :])
    nc.scalar.dma_start(out=msq[0:P - 1, 9:10, :], in_=msq[1:P, 1:2, :])

    # direction classes (int8 0/1): nb0 = sqy < c1*sqx ; nb2 = sqy >= c2*sqx
    # DVE: fused STT on [0:RN].  Pool on [RN:W]: t = c*sqx; d = t - sqy;
    # mask = sign test (exact: f32 subtract has exact sign).
    nb0 = pool.tile([P, 8, W], I8, name="nb0", tag="G")
    nb2 = pool.tile([P, 8, W], I8, name="nb2", tag="Hh")
    sc = pool.tile([P, 8, W - RN], F32, name="sc", tag="SC",
                   padded_shape=[P, 8, W - RX])
    ve.scalar_tensor_tensor(nb0[:, :, 0:RN], sqx_d[:, :, 0:RN], float(c1),
                            sqy_d[:, :, 0:RN], op0=A.mult, op1=A.is_gt)
    ve.scalar_tensor_tensor(nb2[:, :, 0:RN], sqx_d[:, :, 0:RN], float(c2),
                            sqy_d[:, :, 0:RN], op0=A.mult, op1=A.is_le)
    gp.tensor_single_scalar(sc[:, :, :], sqx_d[:, :, RN:W], float(c1),
                            op=A.mult)
    gp.tensor_tensor(sc[:, :, :], sc[:, :, :], sqy_d[:, :, RN:W],
                     op=A.subtract)
    gp.tensor_single_scalar(nb0[:, :, RN:W], sc[:, :, :], 0.0, op=A.is_gt)
    # nb2 Pool side: clobber sqx in place (dead after this)
    gp.tensor_single_scalar(sqx_d[:, :, RN:W], sqx_d[:, :, RN:W], float(c2),
                            op=A.mult)
    gp.tensor_tensor(sqx_d[:, :, RN:W], sqx_d[:, :, RN:W],
                     sqy_d[:, :, RN:W], op=A.subtract)
    gp.tensor_single_scalar(nb2[:, :, RN:W], sqx_d[:, :, RN:W], 0.0,
                            op=A.is_le)

    if stage <= 4:
        bail()
        return

    # ---------------- NMS: directional pair maxes + predicated select ----------
    def msq_sh(dr, dj, c0=0, c1_=W):
        return msq[:, 1 + dr:9 + dr, DOF + dj + c0:DOF + dj + c1_]

    # Pair maxes all on DVE (Pool cannot do TT max, and decomposed
    # sub/relu/add chains stall the pred cascade).  While DVE runs the maxes
    # and the predicated-copy cascade, Pool packs the threshold planes
    # tw/ts = (msq >= t) into 16-bit halfword sums via the pairwise doubling
    # tree (TSS-mult + TT-add, Pool-legal; Act takes the odd-scales).
    # Packing distributes over AND, so afterwards DVE only packs
    # kb = (msq >= M) and bitwise-ANDs the packed words:
    #   pack(kb AND t) = pack(kb) & pack(t)       (bits are independent)
    M = pool.tile([P, 8, W], F32, name="M", tag="B")
    m_d2 = pool.tile([P, 8, W], F32, name="m_d2", tag="A")
    m_ns = pool.tile([P, 8, W], F32, name="m_ns", tag="C")

    # packed halfword-sum planes (f32 sums 0..65535, exact)
    ptw = pool.tile([P, 8, 64], F32, name="ptw", tag="PTW")
    pts = pool.tile([P, 8, 64], F32, name="pts", tag="PTS")

    def pool_pack_tw():
        # quarter-column passes, entirely on Pool (TSS in-place odd-scales +
        # TT adds) so the tree never waits on another engine.
        mults = (2.0, 4.0, 16.0, 256.0)
        for q in range(4):
            th = pool.tile([P, 8, 256], F32, name="th", tag="SC")
            gp.tensor_single_scalar(th[:, :, :],
                                    msq_sh(0, 0, 256 * q, 256 * q + 256),
                                    float(tlow), op=A.is_ge)
            cur = th
            for li in range(4):
                n = 128 >> li
                cur_r = cur.rearrange("p r (j two) -> p r j two", two=2)
                gp.tensor_single_scalar(cur_r[:, :, :, 1], cur_r[:, :, :, 1],
                                        mults[li], op=A.mult)
                if li < 3:
                    nxt = pool.tile([P, 8, n], F32, name=f"pt{li}",
                                    tag=("SD", "SE", "SD")[li])
                    gp.tensor_tensor(nxt[:, :, :], cur_r[:, :, :, 1],
                                     cur_r[:, :, :, 0], op=A.add)
                    cur = nxt
                else:
                    gp.tensor_tensor(ptw[:, :, 16 * q:16 * q + 16],
                                     cur_r[:, :, :, 1], cur_r[:, :, :, 0],
                                     op=A.add)

    def pmax_dve(dst, a_dr, a_dj, b_dr, b_dj):
        ve.tensor_tensor(dst[:, :, :], msq_sh(a_dr, a_dj),
                         msq_sh(b_dr, b_dj), op=A.max)

    pool_pack_tw()
    pmax_dve(m_d2, -1, -1, 1, 1)
    pmax_dve(M, -1, 1, 1, -1)
    HALF = 512
    for c0, c1_ in ((0, HALF), (HALF, W)):
        ve.copy_predicated(M[:, :, c0:c1_], sm[:, :, c0:c1_],
                           m_d2[:, :, c0:c1_])
    m_ew = pool.tile([P, 8, W], F32, name="m_ew", tag="A")
    pmax_dve(m_ns, -1, 0, 1, 0)
    for c0, c1_ in ((0, HALF), (HALF, W)):
        ve.copy_predicated(M[:, :, c0:c1_], nb2[:, :, c0:c1_],
                           m_ns[:, :, c0:c1_])
    pmax_dve(m_ew, 0, 1, 0, -1)
    for c0, c1_ in ((0, HALF), (HALF, W)):
        ve.copy_predicated(M[:, :, c0:c1_], nb0[:, :, c0:c1_],
                           m_ew[:, :, c0:c1_])

    if stage <= 5:
        bail()
        return

    # ---------------- keep-plane pack + AND with threshold planes -------------
    ps = pool.tile([P, HNR, PW], U32, name="ps", tag="tps")
    pw_ = pool.tile([P, HNR, PW], U32, name="pw_", tag="tpw")
    gp.memset(ps[:, :, :], 0)
    gp.memset(pw_[:, :, :], 0)

    # kb = (msq >= M) via exact-sign subtract
    kb = pool.tile([P, 8, W], F32, name="kb", tag="A")
    for eng, c0, c1_ in sp(RA):
        eng.tensor_tensor(kb[:, :, c0:c1_], msq_sh(0, 0, c0, c1_),
                          M[:, :, c0:c1_], op=A.subtract)
    for eng, c0, c1_ in sp(RS):
        eng.tensor_single_scalar(kb[:, :, c0:c1_], kb[:, :, c0:c1_], 0.0,
                                 op=A.is_ge)

    # DVE trees (fused STT levels, per column half with small scratch):
    # one on kb, one on the strong-threshold plane ts
    def dve_tree(plane, out_plane):
        mults = (2.0, 4.0, 16.0, 256.0)
        for hh in (0, 1):
            cur = plane[:, :, 512 * hh:512 * hh + 512]
            n = 256
            tags = ("SC", "SD", "SC")
            for li in range(4):
                cur_r = cur.rearrange("p r (j two) -> p r j two", two=2)
                nxt = (pool.tile([P, 8, n], F32, name=f"kt{li}",
                                 tag=tags[li])
                       if li < 3 else out_plane[:, :, 32 * hh:32 * hh + 32])
                ve.scalar_tensor_tensor(nxt, cur_r[:, :, :, 1], mults[li],
                                        cur_r[:, :, :, 0],
                                        op0=A.mult, op1=A.add)
                cur = nxt
                n //= 2

    ts = pool.tile([P, 8, W], F32, name="ts", tag="B")
    for eng, c0, c1_ in sp(RS):
        eng.tensor_single_scalar(ts[:, :, c0:c1_], msq_sh(0, 0, c0, c1_),
                                 float(thigh), op=A.is_ge)
    pkb = pool.tile([P, 8, 64], F32, name="pkb", tag="PKB")
    dve_tree(kb, pkb)
    dve_tree(ts, pts)

    # convert the three halfword-sum planes to u32 and AND packed words
    ikb = pool.tile([P, 8, 64], U32, name="ikb", tag="SC")
    itw = pool.tile([P, 8, 64], U32, name="itw", tag="SD")
    its = pool.tile([P, 8, 64], U32, name="its", tag="SE")
    ve.tensor_copy(ikb[:, :, :], pkb[:, :, :])
    ve.tensor_copy(itw[:, :, :], ptw[:, :, :])
    ve.tensor_copy(its[:, :, :], pts[:, :, :])
    ve.tensor_tensor(itw[:, :, :], itw[:, :, :], ikb[:, :, :],
                     op=A.bitwise_and)
    ve.tensor_tensor(its[:, :, :], its[:, :, :], ikb[:, :, :],
                     op=A.bitwise_and)
    for t, srcw in ((pw_, itw), (ps, its)):
        hv = srcw.rearrange("p r (s two) -> p r s two", two=2)
        ve.scalar_tensor_tensor(t[:, HOWN:HOWN + 8, 0:NDW], hv[:, :, :, 1],
                                C16A, hv[:, :, :, 0],
                                op0=A.logical_shift_left, op1=A.bitwise_or)

    # ---------------- packed halos ----------------
    def refresh_halos(t):
        nc.sync.dma_start(out=t[1:P, HD0:HD0 + HJ, :],
                          in_=t[0:P - 1, HOWN + 8 - HJ:HOWN + 8, :])
        nc.scalar.dma_start(out=t[0:P - 1, HOWN + 8:HOWN + 8 + HJ, :],
                            in_=t[1:P, HOWN:HOWN + HJ, :])

    refresh_halos(pw_)
    refresh_halos(ps)

    if stage <= 6:
        bail()
        return

    # ---------------- 16 iterations of masked dilation (packed) --------------
    # Bitwise u32 ops are DVE-only on TRN2, so the whole packed loop runs on
    # the vector engine.  V has a zero gutter slot on each side so every
    # word-shift read stays in-tile.  Post-refresh iterations process
    # interior rows first so the halo-exchange DMA latency hides.
    nd = 8 + 2 * HJ
    Vd = pool.tile([P, HNR, NDW + 2], U32, name="Vd", tag="tV")  # gutter,0..31,gutter
    Hd = pool.tile([P, HNR, NDW], U32, name="Hd", tag="tH")
    ve.memset(Vd[:, :, :], 0)
    ve.memset(Hd[:, :, :], 0)

    def rsel(t, g, dr, w0, w1):
        # row-group selector: 'all' rows 1..12, 'core' 4..9, and the two
        # contiguous rim bands (STT ops only accept 2D/3D APs)
        if g == "all":
            return t[:, HD0 + dr:HD0 + nd + dr, w0:w1]
        if g == "core":
            return t[:, 4 + dr:10 + dr, w0:w1]
        if g == "rim1":
            return t[:, 1 + dr:4 + dr, w0:w1]
        if g == "rim2":
            return t[:, 10 + dr:13 + dr, w0:w1]
        if g == "own1":
            return t[:, HOWN + dr:HOWN + 4 + dr, w0:w1]
        return t[:, HOWN + 4 + dr:HOWN + 8 + dr, w0:w1]  # own2

    def hyst_iter(groups=("all",)):
        for g in groups:
            V = rsel(Vd, g, 0, 1, NDW + 1)
            Hh = rsel(Hd, g, 0, 0, NDW)
            ve.tensor_tensor(V, rsel(ps, g, -1, 0, NDW),
                             rsel(ps, g, 1, 0, NDW), op=A.bitwise_or)
            ve.tensor_tensor(V, rsel(ps, g, 0, 0, NDW), V, op=A.bitwise_or)
            ve.scalar_tensor_tensor(Hh, V, C1A, V, op0=A.logical_shift_left,
                                    op1=A.bitwise_or)
            ve.scalar_tensor_tensor(Hh, V, C1A, Hh, op0=A.logical_shift_right,
                                    op1=A.bitwise_or)
            ve.scalar_tensor_tensor(Hh, rsel(Vd, g, 0, 0, NDW), C31A, Hh,
                                    op0=A.logical_shift_right, op1=A.bitwise_or)
            ve.scalar_tensor_tensor(Hh, rsel(Vd, g, 0, 2, NDW + 2), C31A, Hh,
                                    op0=A.logical_shift_left, op1=A.bitwise_or)
        for g in groups:
            ve.tensor_tensor(rsel(ps, g, 0, 0, NDW), rsel(Hd, g, 0, 0, NDW),
                             rsel(pw_, g, 0, 0, NDW), op=A.bitwise_and)

    n_iters = int(os.environ.get("CANNY_HYST_ITERS", 16))
    no_refresh = int(os.environ.get("CANNY_NO_REFRESH", 0))  # timing expt only
    hide = int(os.environ.get("CANNY_HIDE_REFRESH", 1))
    for it in range(n_iters):
        post_refresh = hide and it > 0 and it % HJ == 0 and not no_refresh
        if it == n_iters - 1:
            # final iteration: halo rows are dead afterwards; two own-row
            # halves let the unpack start on half 1 while half 2 runs
            hyst_iter(("own1", "own2"))
        else:
            hyst_iter(("core", "rim1", "rim2") if post_refresh
                      else ("all",))
        if (it + 1) % HJ == 0 and it < n_iters - 1 and not no_refresh:
            refresh_halos(ps)

    if stage <= 7:
        bail()
        return

    # ---------------- unpack own rows -> f32 0/1 and store --------------------
    # ps own words viewed as u16 halfwords h (= pixels 16h..16h+15).  For
    # each bit position k: tub[k][h] = hw[h] << (15-k) (single-src imm-shift
    # TSS on packed u16 = 4x DVE mode), then sign test in the transposed
    # view.  Processed in row halves so the store overlaps the unpack.
    own_hw = ps[:, HOWN:HOWN + 8, 0:NDW].bitcast(U16)   # [P, 8, 64]
    outf = pool.tile([P, 8, W], F32, name="outf", tag="B")
    outf_r = outf.rearrange("p r (h k) -> p r h k", k=16)
    out_r = out_d.rearrange("(p r w) -> p r w", p=P, r=R)
    for hf, (r0, r1) in enumerate(((0, 4), (4, 8))):
        tub = pool.tile([P, 4, 16, 64], U16, name=f"tub{hf}",
                        tag="SC")
        for k in range(16):
            ve.tensor_single_scalar(tub[:, :, k, :], own_hw[:, r0:r1, :],
                                    15 - k, op=A.logical_shift_left)
        tub_px = tub.bitcast(I16).rearrange("p r k h -> p r h k")
        ve.tensor_single_scalar(outf_r[:, r0:r1, :, :], tub_px[:, :, :, :],
                                0, op=A.is_lt)
        nc.sync.dma_start(out=out_r[:, r0:r1, :], in_=outf[:, r0:r1, :])


_CACHE = {}


def _get_built():
    if "nc" not in _CACHE:
        from concourse import bacc
        nc = bacc.Bacc(None)
        img_d = nc.declare_dram_parameter("img", [H * W], F32, isOutput=False)
        out_d = nc.declare_dram_parameter("out", [H * W], F32, isOutput=True)
        with TileContext(nc) as tc:
            with tc.tile_pool(name="main", bufs=1) as pool:
                build_canny(nc, tc, pool, img_d, out_d)
        nc.finalize()
        _CACHE["nc"] = nc
    return _CACHE["nc"]


TRACE = False        # set True (e.g. from test.py) to capture an NTFF profile
LAST_RESULT = None   # BassKernelResults of the most recent run


def kernel(image):
    global LAST_RESULT
    image = np.ascontiguousarray(np.asarray(image), dtype=np.float32)
    B = image.shape[0]
    assert image.shape == (B, 1, H, W)
    nc = _get_built()
    in_maps = [{"img": image[i, 0].reshape(-1)} for i in range(B)]
    res = run_bass_kernel_spmd(nc, in_maps, core_ids=list(range(B)),
                               trace=TRACE)
    LAST_RESULT = res
    out = np.stack([r["out"].reshape(H, W) for r in res.results])
    return out[:, None].astype(np.float32)


# revision 46
# speedup vs baseline: 1.0087x; 1.0012x over previous
"""Canny edge detector on 8 Trainium2 NeuronCores — pure data-parallel (1 image/core).

Pipeline per core (image 1024x1024 f32):
  1. 5x5 Gaussian blur (separable: vertical then horizontal 5-tap, exact f32)
  2. Sobel gx, gy (separable 3-taps)
  3. NMS using squared magnitudes (no sqrt / atan2 needed)
  4. Hysteresis: 16 iterations of 3x3 binary dilation masked by weak, on
     bit-packed state (32 px/word) with per-row gutter words.

Layout: "multirow" — partition p holds image rows [8p+d] in its free
dimension, row pitch 1028 (2 zero gutter cols each side) so ALL 8-neighbor
shifts are free-dim AP offsets.  Vertical halos come from overlapping HBM
loads (img) and SBUF->SBUF DMA halo refreshes (blurred, msq, packed state).

Engine facts (BIR-verifier-probed): Pool/GPSIMD supports ONLY f32
tensor_tensor add/sub/mult, tensor_single_scalar mult/add/max/compares,
copy, memset, iota.  No STT, no TT max/compare, no bitwise/shifts.
DVE does everything; Act does single-input activations (copy-scale,
square, relu).  So:
  - TT add/sub/mult ops column-split DVE|Pool at 672 (rates 1.042 vs 1.984)
  - single-scalar ops split at 745 (0.521 vs 1.389)
  - fused STT combines (a*s + b) run on DVE for cols [0:782], decomposed
    TSS-mult + TT-add on Pool for the rest
  - NMS pair maxes: DVE TT-max [0:810], Pool sub + Act relu + Pool add
    beyond (max(a,b) = b + relu(a-b); <=1-ulp rounding, flips only exact
    NMS ties — probability ~1e-12 per pixel)
  - hysteresis bitwise loop is DVE-only (hardware restriction), with
    interior-first iterations after each halo exchange to hide DMA latency
"""
import numpy as np

import concourse.bass as bass
import concourse.mybir as mybir
from concourse.tile import TileContext
from concourse.bass_utils import run_bass_kernel_spmd

P = 128          # partitions
R = 8            # image rows per partition
H = W = 1024
RP = 1028        # row pitch (2 gutter cols + 1024 data + 2 gutter cols)
DOF = 2          # data column offset within a row slot

# packed layout: 32 px/word -> 32 data words + 1 zero gutter word per row
PW = 33
NDW = 32

# hysteresis packed tile: 1 margin + (J halo + 8 own + J halo) data rows + 1 margin
HJ = 2           # halo rows == refresh cadence (iterations between halo refreshes)
HNR = 2 + 8 + 2 * HJ
HD0 = 1          # first data row (halo-top) in packed tiles
HOWN = 1 + HJ    # first own row in packed tiles

F32 = mybir.dt.float32
U32 = mybir.dt.uint32
U16 = mybir.dt.uint16
I16 = mybir.dt.int16
I32 = mybir.dt.int32
I8 = mybir.dt.int8

# DVE | Pool column splits (DVE gets [0:r), Pool [r:W))
RA = 672     # TT add/sub/mult      (DVE 1.042 vs Pool 1.984 ns/elem)
RS = 745     # single-scalar ops    (DVE 0.521 vs Pool 1.389)
RC = 782     # STT combine vs Pool TSS-mult + TT-add  (1.042 vs 3.373)
RX = 810     # TT max vs Pool sub + Act relu + Pool add (1.042 vs 3.968)
RN = 840     # STT cmp vs Pool TSS-mult + TT-sub + TSS-cmp (1.042 vs 4.762)
RT = 390     # pack-tree level-1 split of 512 pair sums


def _f32_consts():
    ax = np.arange(5, dtype=np.float32) - np.float32(2.0)
    g = np.exp(-(ax ** 2) / np.float32(2.0)).astype(np.float32)
    g = (g / g.sum()).astype(np.float32)
    c1 = np.float32(np.tan(np.deg2rad(22.5)) ** 2)
    c2 = np.float32(np.tan(np.deg2rad(67.5)) ** 2)

    def sqrt_thresh(t):
        t = np.float32(t)
        x = np.float32(t) * np.float32(t)
        while np.sqrt(np.float32(x)) >= t:
            x = np.nextafter(x, np.float32(0.0), dtype=np.float32)
        while np.sqrt(np.float32(x)) < t:
            x = np.nextafter(x, np.float32(np.inf), dtype=np.float32)
        return np.float32(x)

    return g, c1, c2, sqrt_thresh(0.1), sqrt_thresh(0.2)


def build_canny(nc, tc, pool, img_d, out_d, stage=99):
    import os
    stage = int(os.environ.get("CANNY_STAGE", stage))
    from concourse.alu_op_type import AluOpType as A
    g, c1, c2, tlow, thigh = _f32_consts()
    ve = nc.vector
    gp = nc.gpsimd
    se = nc.scalar

    def bail():
        z = pool.tile([P, 8, W], F32, name="zz", tag="C")
        ve.memset(z[:, :, :], 0.0)
        nc.sync.dma_start(out=out_d.rearrange("(p r w) -> p r w", p=P, r=R),
                          in_=z[:, :, :])

    def sp(r):
        return ((ve, 0, r), (gp, r, W))

    def zero_gutters(eng, t, nr):
        eng.memset(t[:, 0:nr, 0:DOF], 0.0)
        eng.memset(t[:, 0:nr, DOF + W:RP], 0.0)

    def comb(mk_dst, mk_src, s):
        """dst += s*src: DVE fused STT on [0:RC]; Pool scales src in place
        (src must be dead afterwards) then adds, on [RC:W]."""
        ve.scalar_tensor_tensor(mk_dst(0, RC), mk_src(0, RC), float(s),
                                mk_dst(0, RC), op0=A.mult, op1=A.add)
        gp.tensor_single_scalar(mk_src(RC, W), mk_src(RC, W), float(s),
                                op=A.mult)
        gp.tensor_tensor(mk_dst(RC, W), mk_src(RC, W), mk_dst(RC, W),
                         op=A.add)

    def comb_act(mk_dst, mk_src, s):
        """dst += s*src with the scale on Act (in place over src, which must
        be dead afterwards) and the add column-split DVE|Pool.  Same two
        roundings as the fused STT path — bit-identical."""
        se.mul(mk_src(0, W), mk_src(0, W), float(s))
        for eng, c0, c1_ in sp(RA):
            eng.tensor_tensor(mk_dst(c0, c1_), mk_src(c0, c1_),
                              mk_dst(c0, c1_), op=A.add)

    # per-partition integer scalar constants for bitwise scalar_tensor_tensor
    cst = pool.tile([P, 4], U32, name="cst", tag="tcst")
    ve.memset(cst[:, 0:1], 1)
    ve.memset(cst[:, 1:2], 16)
    ve.memset(cst[:, 2:3], 31)
    C1A, C16A, C31A = cst[:, 0:1], cst[:, 1:2], cst[:, 2:3]

    # ---------------- load image (rows 8p-2 .. 8p+10) ----------------
    img = pool.tile([P, 12, W], F32, name="img", tag="A")
    # out-of-image halo rows must be zero; the loads below overwrite all but
    # partition 0 / 127 edges (compute ops cannot start at partition 127, so
    # full-partition memsets, one per engine, before the loads)
    ve.memset(img[:, 0:2, :], 0.0)
    gp.memset(img[:, 10:12, :], 0.0)  # full partitions: p127 can't be sliced

    img_rows = img_d.rearrange("(n w) -> n w", w=W)
    # small partition-edge loads first, then the main window in column
    # halves (left half feeds the DVE shares of the first blur ops, so
    # compute starts before the right half lands)
    nc.sync.dma_start(out=img[0:1, 2:12, :],
                      in_=img_rows[0:10, :].rearrange("(p r) w -> p r w", p=1))
    nc.sync.dma_start(out=img[P - 1:P, 0:10, :],
                      in_=img_rows[H - 10:H, :].rearrange("(p r) w -> p r w", p=1))
    for c0, c1_ in ((0, RA), (RA, W)):
        img_win = bass.AP(img_d, (R - 2) * W + c0,
                          [[R * W, P - 2], [W, 12], [1, c1_ - c0]])
        nc.sync.dma_start(out=img[1:P - 1, :, c0:c1_], in_=img_win)

    # ---------------- vertical 5-tap blur -> blurv (own 8 rows) ----------------
    blurv = pool.tile([P, 8, RP], F32, name="blurv", tag="B")
    zero_gutters(ve, blurv, 8)
    pa1 = pool.tile([P, 8, W], F32, name="pa1", tag="C")
    pa2 = pool.tile([P, 8, W], F32, name="pa2", tag="F")
    for eng, c0, c1_ in sp(RA):
        eng.tensor_tensor(pa1[:, :, c0:c1_], img[:, 1:9, c0:c1_],
                          img[:, 3:11, c0:c1_], op=A.add)
        eng.tensor_tensor(pa2[:, :, c0:c1_], img[:, 0:8, c0:c1_],
                          img[:, 4:12, c0:c1_], op=A.add)
    se.mul(blurv[:, :, DOF:DOF + RA], img[:, 2:10, 0:RA], float(g[2]))
    se.mul(blurv[:, :, DOF + RA:DOF + W], img[:, 2:10, RA:W], float(g[2]))
    comb(lambda a, b: blurv[:, :, DOF + a:DOF + b],
         lambda a, b: pa1[:, :, a:b], g[1])
    comb(lambda a, b: blurv[:, :, DOF + a:DOF + b],
         lambda a, b: pa2[:, :, a:b], g[0])

    if stage <= 1:
        bail()
        return

    # ---------------- horizontal 5-tap blur -> blurred [10 rows, own at 1..9] ---
    blurred = pool.tile([P, 10, RP], F32, name="blurred", tag="A")
    pb1 = pool.tile([P, 8, W], F32, name="pb1", tag="C")
    pb2 = pool.tile([P, 8, W], F32, name="pb2", tag="F")
    for eng, c0, c1_ in sp(RA):
        eng.tensor_tensor(pb1[:, :, c0:c1_],
                          blurv[:, :, DOF + c0 - 1:DOF + c1_ - 1],
                          blurv[:, :, DOF + c0 + 1:DOF + c1_ + 1], op=A.add)
        eng.tensor_tensor(pb2[:, :, c0:c1_],
                          blurv[:, :, DOF + c0 - 2:DOF + c1_ - 2],
                          blurv[:, :, DOF + c0 + 2:DOF + c1_ + 2], op=A.add)
    se.mul(blurred[:, 1:9, DOF:DOF + W], blurv[:, :, DOF:DOF + W], float(g[2]))
    comb(lambda a, b: blurred[:, 1:9, DOF + a:DOF + b],
         lambda a, b: pb1[:, :, a:b], g[1])
    comb(lambda a, b: blurred[:, 1:9, DOF + a:DOF + b],
         lambda a, b: pb2[:, :, a:b], g[0])
    # halo refresh: row 0 <- p-1 own row 7 (tile row 8); row 9 <- p+1 own row 0
    ve.memset(blurred[:, 0:1, :], 0.0)
    ve.memset(blurred[:, 9:10, :], 0.0)
    nc.sync.dma_start(out=blurred[1:P, 0:1, DOF:DOF + W],
                      in_=blurred[0:P - 1, 8:9, DOF:DOF + W])
    nc.scalar.dma_start(out=blurred[0:P - 1, 9:10, DOF:DOF + W],
                        in_=blurred[1:P, 1:2, DOF:DOF + W])

    if stage <= 2:
        bail()
        return

    # ---------------- sobel vertical parts (own 8 rows) ----------------
    # wx = bl[r-1] + 2 bl[r] + bl[r+1] ; vy = bl[r+1] - bl[r-1]
    wx = pool.tile([P, 8, RP], F32, name="wx", tag="C")
    vy = pool.tile([P, 8, RP], F32, name="vy", tag="F")
    zero_gutters(ve, wx, 8)
    zero_gutters(gp, vy, 8)
    for eng, c0, c1_ in sp(RA):
        eng.tensor_tensor(wx[:, :, DOF + c0:DOF + c1_],
                          blurred[:, 0:8, DOF + c0:DOF + c1_],
                          blurred[:, 2:10, DOF + c0:DOF + c1_], op=A.add)
        eng.tensor_tensor(vy[:, :, DOF + c0:DOF + c1_],
                          blurred[:, 2:10, DOF + c0:DOF + c1_],
                          blurred[:, 0:8, DOF + c0:DOF + c1_], op=A.subtract)
    # wx += 2*bl(center); Pool side scales blurred rows 1:9 in place (dead after)
    comb(lambda a, b: wx[:, :, DOF + a:DOF + b],
         lambda a, b: blurred[:, 1:9, DOF + a:DOF + b], 2.0)

    # ---------------- sobel horizontal parts ----------------
    gx = pool.tile([P, 8, RP], F32, name="gx", tag="B")
    gy = pool.tile([P, 8, RP], F32, name="gy", tag="A")
    gx_d = gx[:, :, DOF:DOF + W]
    gy_d = gy[:, :, DOF:DOF + W]
    for eng, c0, c1_ in sp(RA):
        eng.tensor_tensor(gx[:, :, DOF + c0:DOF + c1_],
                          wx[:, :, DOF + c0 + 1:DOF + c1_ + 1],
                          wx[:, :, DOF + c0 - 1:DOF + c1_ - 1], op=A.subtract)
        eng.tensor_tensor(gy[:, :, DOF + c0:DOF + c1_],
                          vy[:, :, DOF + c0 - 1:DOF + c1_ - 1],
                          vy[:, :, DOF + c0 + 1:DOF + c1_ + 1], op=A.add)
    # gy += 2*vy; Pool side scales vy in place (dead after)
    comb(lambda a, b: gy[:, :, DOF + a:DOF + b],
         lambda a, b: vy[:, :, DOF + a:DOF + b], 2.0)

    if stage <= 3:
        bail()
        return

    # ---------------- sign of gx*gy, squares, msq ----------------
    # diagonal-class mask: sm = (gx*gy < 0).  Product underflow to +-0 only
    # happens when msq is far below the weak threshold, where the NMS
    # direction choice can't affect the output.
    smw = pool.tile([P, 8, W], F32, name="smw", tag="C")
    sm = pool.tile([P, 8, W], I8, name="sm", tag="G2")
    for eng, c0, c1_ in sp(RA):
        eng.tensor_tensor(smw[:, :, c0:c1_], gx[:, :, DOF + c0:DOF + c1_],
                          gy[:, :, DOF + c0:DOF + c1_], op=A.mult)
    for eng, c0, c1_ in sp(RS):
        eng.tensor_single_scalar(sm[:, :, c0:c1_], smw[:, :, c0:c1_], 0.0,
                                 op=A.is_lt)

    se.square(gx_d, gx_d)   # sqx
    se.square(gy_d, gy_d)   # sqy
    sqx, sqy = gx, gy
    sqx_d, sqy_d = gx_d, gy_d

    # msq [10 rows, own at 1..9] with DMA halo refresh (before nb0/nb2 so the
    # Pool decompositions may clobber sqx afterwards)
    msq = pool.tile([P, 10, RP], F32, name="msq", tag="F")
    zero_gutters(ve, msq, 10)
    for eng, c0, c1_ in sp(RA):
        eng.tensor_tensor(msq[:, 1:9, DOF + c0:DOF + c1_],
                          sqx[:, :, DOF + c0:DOF + c1_],
                          sqy[:, :, DOF + c0:DOF + c1_], op=A.add)
    ve.memset(msq[:, 0:1, :], 0.0)
    ve.memset(msq[:, 9:10, :], 0.0)
    nc.sync.dma_start(out=msq[1:P, 0:1, :], in_=msq[0:P - 1, 8:9, :])
    nc.scalar.dma_start(out=msq[0:P - 1, 9:10, :], in_=msq[1:P, 1:2, :])

    # direction classes (int8 0/1): nb0 = sqy < c1*sqx ; nb2 = sqy >= c2*sqx
    # DVE: fused STT on [0:RN].  Pool on [RN:W]: t = c*sqx; d = t - sqy;
    # mask = sign test (exact: f32 subtract has exact sign).
    nb0 = pool.tile([P, 8, W], I8, name="nb0", tag="G")
    nb2 = pool.tile([P, 8, W], I8, name="nb2", tag="Hh")
    sc = pool.tile([P, 8, W - RN], F32, name="sc", tag="SC",
                   padded_shape=[P, 8, W - RX])
    ve.scalar_tensor_tensor(nb0[:, :, 0:RN], sqx_d[:, :, 0:RN], float(c1),
                            sqy_d[:, :, 0:RN], op0=A.mult, op1=A.is_gt)
    ve.scalar_tensor_tensor(nb2[:, :, 0:RN], sqx_d[:, :, 0:RN], float(c2),
                            sqy_d[:, :, 0:RN], op0=A.mult, op1=A.is_le)
    gp.tensor_single_scalar(sc[:, :, :], sqx_d[:, :, RN:W], float(c1),
                            op=A.mult)
    gp.tensor_tensor(sc[:, :, :], sc[:, :, :], sqy_d[:, :, RN:W],
                     op=A.subtract)
    gp.tensor_single_scalar(nb0[:, :, RN:W], sc[:, :, :], 0.0, op=A.is_gt)
    # nb2 Pool side: clobber sqx in place (dead after this)
    gp.tensor_single_scalar(sqx_d[:, :, RN:W], sqx_d[:, :, RN:W], float(c2),
                            op=A.mult)
    gp.tensor_tensor(sqx_d[:, :, RN:W], sqx_d[:, :, RN:W],
                     sqy_d[:, :, RN:W], op=A.subtract)
    gp.tensor_single_scalar(nb2[:, :, RN:W], sqx_d[:, :, RN:W], 0.0,
                            op=A.is_le)

    if stage <= 4:
        bail()
        return

    # ---------------- NMS: directional pair maxes + predicated select ----------
    def msq_sh(dr, dj, c0=0, c1_=W):
        return msq[:, 1 + dr:9 + dr, DOF + dj + c0:DOF + dj + c1_]

    # Pair maxes all on DVE (Pool cannot do TT max, and decomposed
    # sub/relu/add chains stall the pred cascade).  While DVE runs the maxes
    # and the predicated-copy cascade, Pool packs the threshold planes
    # tw/ts = (msq >= t) into 16-bit halfword sums via the pairwise doubling
    # tree (TSS-mult + TT-add, Pool-legal; Act takes the odd-scales).
    # Packing distributes over AND, so afterwards DVE only packs
    # kb = (msq >= M) and bitwise-ANDs the packed words:
    #   pack(kb AND t) = pack(kb) & pack(t)       (bits are independent)
    M = pool.tile([P, 8, W], F32, name="M", tag="B")
    m_d2 = pool.tile([P, 8, W], F32, name="m_d2", tag="A")
    m_ns = pool.tile([P, 8, W], F32, name="m_ns", tag="C")

    # packed halfword-sum planes (f32 sums 0..65535, exact)
    ptw = pool.tile([P, 8, 64], F32, name="ptw", tag="PTW")
    pts = pool.tile([P, 8, 64], F32, name="pts", tag="PTS")

    def pool_pack_tw():
        # quarter-column passes, entirely on Pool (TSS in-place odd-scales +
        # TT adds) so the tree never waits on another engine.
        mults = (2.0, 4.0, 16.0, 256.0)
        for q in range(4):
            th = pool.tile([P, 8, 256], F32, name="th", tag="SC")
            gp.tensor_single_scalar(th[:, :, :],
                                    msq_sh(0, 0, 256 * q, 256 * q + 256),
                                    float(tlow), op=A.is_ge)
            cur = th
            for li in range(4):
                n = 128 >> li
                cur_r = cur.rearrange("p r (j two) -> p r j two", two=2)
                gp.tensor_single_scalar(cur_r[:, :, :, 1], cur_r[:, :, :, 1],
                                        mults[li], op=A.mult)
                if li < 3:
                    nxt = pool.tile([P, 8, n], F32, name=f"pt{li}",
                                    tag=("SD", "SE", "SD")[li])
                    gp.tensor_tensor(nxt[:, :, :], cur_r[:, :, :, 1],
                                     cur_r[:, :, :, 0], op=A.add)
                    cur = nxt
                else:
                    gp.tensor_tensor(ptw[:, :, 16 * q:16 * q + 16],
                                     cur_r[:, :, :, 1], cur_r[:, :, :, 0],
                                     op=A.add)

    def pmax_dve(dst, a_dr, a_dj, b_dr, b_dj):
        ve.tensor_tensor(dst[:, :, :], msq_sh(a_dr, a_dj),
                         msq_sh(b_dr, b_dj), op=A.max)

    pool_pack_tw()
    pmax_dve(m_d2, -1, -1, 1, 1)
    pmax_dve(M, -1, 1, 1, -1)
    HALF = 512
    for c0, c1_ in ((0, HALF), (HALF, W)):
        ve.copy_predicated(M[:, :, c0:c1_], sm[:, :, c0:c1_],
                           m_d2[:, :, c0:c1_])
    m_ew = pool.tile([P, 8, W], F32, name="m_ew", tag="A")
    pmax_dve(m_ns, -1, 0, 1, 0)
    for c0, c1_ in ((0, HALF), (HALF, W)):
        ve.copy_predicated(M[:, :, c0:c1_], nb2[:, :, c0:c1_],
                           m_ns[:, :, c0:c1_])
    pmax_dve(m_ew, 0, 1, 0, -1)
    for c0, c1_ in ((0, HALF), (HALF, W)):
        ve.copy_predicated(M[:, :, c0:c1_], nb0[:, :, c0:c1_],
                           m_ew[:, :, c0:c1_])

    if stage <= 5:
        bail()
        return

    # ---------------- keep-plane pack + AND with threshold planes -------------
    ps = pool.tile([P, HNR, PW], U32, name="ps", tag="tps")
    pw_ = pool.tile([P, HNR, PW], U32, name="pw_", tag="tpw")
    gp.memset(ps[:, :, :], 0)
    gp.memset(pw_[:, :, :], 0)

    # kb = (msq >= M) via exact-sign subtract
    kb = pool.tile([P, 8, W], F32, name="kb", tag="A")
    for eng, c0, c1_ in sp(RA):
        eng.tensor_tensor(kb[:, :, c0:c1_], msq_sh(0, 0, c0, c1_),
                          M[:, :, c0:c1_], op=A.subtract)
    for eng, c0, c1_ in sp(RS):
        eng.tensor_single_scalar(kb[:, :, c0:c1_], kb[:, :, c0:c1_], 0.0,
                                 op=A.is_ge)

    # DVE trees (fused STT levels, per column half with small scratch):
    # one on kb, one on the strong-threshold plane ts
    def dve_tree(plane, out_plane):
        mults = (2.0, 4.0, 16.0, 256.0)
        for hh in (0, 1):
            cur = plane[:, :, 512 * hh:512 * hh + 512]
            n = 256
            tags = ("SC", "SD", "SC")
            for li in range(4):
                cur_r = cur.rearrange("p r (j two) -> p r j two", two=2)
                nxt = (pool.tile([P, 8, n], F32, name=f"kt{li}",
                                 tag=tags[li])
                       if li < 3 else out_plane[:, :, 32 * hh:32 * hh + 32])
                ve.scalar_tensor_tensor(nxt, cur_r[:, :, :, 1], mults[li],
                                        cur_r[:, :, :, 0],
                                        op0=A.mult, op1=A.add)
                cur = nxt
                n //= 2

    ts = pool.tile([P, 8, W], F32, name="ts", tag="B")
    for eng, c0, c1_ in sp(RS):
        eng.tensor_single_scalar(ts[:, :, c0:c1_], msq_sh(0, 0, c0, c1_),
                                 float(thigh), op=A.is_ge)
    pkb = pool.tile([P, 8, 64], F32, name="pkb", tag="PKB")
    dve_tree(kb, pkb)
    dve_tree(ts, pts)

    # convert the three halfword-sum planes to u32 and AND packed words
    ikb = pool.tile([P, 8, 64], U32, name="ikb", tag="SC")
    itw = pool.tile([P, 8, 64], U32, name="itw", tag="SD")
    its = pool.tile([P, 8, 64], U32, name="its", tag="SE")
    ve.tensor_copy(ikb[:, :, :], pkb[:, :, :])
    ve.tensor_copy(itw[:, :, :], ptw[:, :, :])
    ve.tensor_copy(its[:, :, :], pts[:, :, :])
    ve.tensor_tensor(itw[:, :, :], itw[:, :, :], ikb[:, :, :],
                     op=A.bitwise_and)
    ve.tensor_tensor(its[:, :, :], its[:, :, :], ikb[:, :, :],
                     op=A.bitwise_and)
    for t, srcw in ((pw_, itw), (ps, its)):
        hv = srcw.rearrange("p r (s two) -> p r s two", two=2)
        ve.scalar_tensor_tensor(t[:, HOWN:HOWN + 8, 0:NDW], hv[:, :, :, 1],
                                C16A, hv[:, :, :, 0],
                                op0=A.logical_shift_left, op1=A.bitwise_or)

    # ---------------- packed halos ----------------
    def refresh_halos(t):
        nc.sync.dma_start(out=t[1:P, HD0:HD0 + HJ, :],
                          in_=t[0:P - 1, HOWN + 8 - HJ:HOWN + 8, :])
        nc.scalar.dma_start(out=t[0:P - 1, HOWN + 8:HOWN + 8 + HJ, :],
                            in_=t[1:P, HOWN:HOWN + HJ, :])

    refresh_halos(pw_)
    refresh_halos(ps)

    if stage <= 6:
        bail()
        return

    # ---------------- 16 iterations of masked dilation (packed) --------------
    # Bitwise u32 ops are DVE-only on TRN2, so the whole packed loop runs on
    # the vector engine.  V has a zero gutter slot on each side so every
    # word-shift read stays in-tile.  Post-refresh iterations process
    # interior rows first so the halo-exchange DMA latency hides.
    nd = 8 + 2 * HJ
    Vd = pool.tile([P, HNR, NDW + 2], U32, name="Vd", tag="tV")  # gutter,0..31,gutter
    Hd = pool.tile([P, HNR, NDW], U32, name="Hd", tag="tH")
    ve.memset(Vd[:, :, :], 0)
    ve.memset(Hd[:, :, :], 0)

    def rsel(t, g, dr, w0, w1):
        # row-group selector: 'all' rows 1..12, 'core' 4..9, and the two
        # contiguous rim bands (STT ops only accept 2D/3D APs)
        if g == "all":
            return t[:, HD0 + dr:HD0 + nd + dr, w0:w1]
        if g == "core":
            return t[:, 4 + dr:10 + dr, w0:w1]
        if g == "rim1":
            return t[:, 1 + dr:4 + dr, w0:w1]
        if g == "rim2":
            return t[:, 10 + dr:13 + dr, w0:w1]
        if g == "own1":
            return t[:, HOWN + dr:HOWN + 4 + dr, w0:w1]
        if g == "own2":
            return t[:, HOWN + 4 + dr:HOWN + 8 + dr, w0:w1]
        if g == "rim1p":
            return t[:, 2 + dr:4 + dr, w0:w1]
        return t[:, 10 + dr:12 + dr, w0:w1]  # rim2p

    def hyst_iter(groups=("all",)):
        for g in groups:
            V = rsel(Vd, g, 0, 1, NDW + 1)
            Hh = rsel(Hd, g, 0, 0, NDW)
            ve.tensor_tensor(V, rsel(ps, g, -1, 0, NDW),
                             rsel(ps, g, 1, 0, NDW), op=A.bitwise_or)
            ve.tensor_tensor(V, rsel(ps, g, 0, 0, NDW), V, op=A.bitwise_or)
            ve.scalar_tensor_tensor(Hh, V, C1A, V, op0=A.logical_shift_left,
                                    op1=A.bitwise_or)
            ve.scalar_tensor_tensor(Hh, V, C1A, Hh, op0=A.logical_shift_right,
                                    op1=A.bitwise_or)
            ve.scalar_tensor_tensor(Hh, rsel(Vd, g, 0, 0, NDW), C31A, Hh,
                                    op0=A.logical_shift_right, op1=A.bitwise_or)
            ve.scalar_tensor_tensor(Hh, rsel(Vd, g, 0, 2, NDW + 2), C31A, Hh,
                                    op0=A.logical_shift_left, op1=A.bitwise_or)
        for g in groups:
            ve.tensor_tensor(rsel(ps, g, 0, 0, NDW), rsel(Hd, g, 0, 0, NDW),
                             rsel(pw_, g, 0, 0, NDW), op=A.bitwise_and)

    n_iters = int(os.environ.get("CANNY_HYST_ITERS", 16))
    no_refresh = int(os.environ.get("CANNY_NO_REFRESH", 0))  # timing expt only
    hide = int(os.environ.get("CANNY_HIDE_REFRESH", 1))
    for it in range(n_iters):
        post_refresh = hide and it > 0 and it % HJ == 0 and not no_refresh
        if it == n_iters - 1:
            # final iteration: halo rows are dead afterwards; two own-row
            # halves let the unpack start on half 1 while half 2 runs
            hyst_iter(("own1", "own2"))
        elif it == n_iters - 2:
            # penultimate iteration: the outermost halo rows (1, 12) are
            # dead too — the final iteration reads ps rows 2..11 only
            hyst_iter(("core", "rim1p", "rim2p") if post_refresh
                      else ("core", "rim1p", "rim2p"))
        else:
            hyst_iter(("core", "rim1", "rim2") if post_refresh
                      else ("all",))
        if (it + 1) % HJ == 0 and it < n_iters - 1 and not no_refresh:
            refresh_halos(ps)

    if stage <= 7:
        bail()
        return

    # ---------------- unpack own rows -> f32 0/1 and store --------------------
    # ps own words viewed as u16 halfwords h (= pixels 16h..16h+15).  For
    # each bit position k: tub[k][h] = hw[h] << (15-k) (single-src imm-shift
    # TSS on packed u16 = 4x DVE mode), then sign test in the transposed
    # view.  Processed in row halves so the store overlaps the unpack.
    own_hw = ps[:, HOWN:HOWN + 8, 0:NDW].bitcast(U16)   # [P, 8, 64]
    outf = pool.tile([P, 8, W], F32, name="outf", tag="B")
    outf_r = outf.rearrange("p r (h k) -> p r h k", k=16)
    out_r = out_d.rearrange("(p r w) -> p r w", p=P, r=R)
    for hf, (r0, r1) in enumerate(((0, 4), (4, 8))):
        tub = pool.tile([P, 4, 16, 64], U16, name=f"tub{hf}",
                        tag="SC")
        for k in range(16):
            ve.tensor_single_scalar(tub[:, :, k, :], own_hw[:, r0:r1, :],
                                    15 - k, op=A.logical_shift_left)
        tub_px = tub.bitcast(I16).rearrange("p r k h -> p r h k")
        ve.tensor_single_scalar(outf_r[:, r0:r1, :, :], tub_px[:, :, :, :],
                                0, op=A.is_lt)
        nc.sync.dma_start(out=out_r[:, r0:r1, :], in_=outf[:, r0:r1, :])


_CACHE = {}


def _get_built():
    if "nc" not in _CACHE:
        from concourse import bacc
        nc = bacc.Bacc(None)
        img_d = nc.declare_dram_parameter("img", [H * W], F32, isOutput=False)
        out_d = nc.declare_dram_parameter("out", [H * W], F32, isOutput=True)
        with TileContext(nc) as tc:
            with tc.tile_pool(name="main", bufs=1) as pool:
                build_canny(nc, tc, pool, img_d, out_d)
        nc.finalize()
        _CACHE["nc"] = nc
    return _CACHE["nc"]


TRACE = False        # set True (e.g. from test.py) to capture an NTFF profile
LAST_RESULT = None   # BassKernelResults of the most recent run


def kernel(image):
    global LAST_RESULT
    image = np.ascontiguousarray(np.asarray(image), dtype=np.float32)
    B = image.shape[0]
    assert image.shape == (B, 1, H, W)
    nc = _get_built()
    in_maps = [{"img": image[i, 0].reshape(-1)} for i in range(B)]
    res = run_bass_kernel_spmd(nc, in_maps, core_ids=list(range(B)),
                               trace=TRACE)
    LAST_RESULT = res
    out = np.stack([r["out"].reshape(H, W) for r in res.results])
    return out[:, None].astype(np.float32)
